# revision 24
# baseline (speedup 1.0000x reference)
"""Trainium2 Bass kernel for AudioOnlyGNN (3-layer GCN + BatchNorm + mean-pool + MLP).

Structure (v2 — "static slot stream" design):

Nodes are renumbered by degree (host-side, pure index manipulation) and dealt
round-robin to the 8 cores in 128-row tiles, so that every local tile t holds
nodes of near-identical in-degree.  Each tile gets a uniform per-node slot
budget k_t = max in-degree(+self) over that tile across all cores, giving a
*static* slot stream of 128*k_t slots per tile (identical shape on every
core).  For each layer, the host materialises the edge-source rows in slot
order (a pure gather / data movement step, like the baseline's inter-launch
tile_major permutation) so the device reads them with large contiguous DMA
descriptors instead of per-edge gather descriptors.

On device, a 128-slot block contributes to a [F, 128] PSUM tile via a single
matmul whose moving operand is a small static "panel" matrix (slot -> dst
column weight, the GCN normalisation coefficients baked in by the host from
the graph structure).  The per-tile aggregate is then transformed
(W^T @ agg -> [H, dst]) with bias/BN-shift added as rank-1 matmuls, ReLU'd,
and written back.  Layers 0/1 write h'[dst] = dinv[dst]*ReLU(...) (folded
into the panel weights of the next layer), so panels never depend on h.
Tiles are processed in pairs sharing [128, 256] PSUM tiles so the
PSUM->SBUF copies and ReLUs are batched; the PSUM reset is one matmul
against a zero row, which lets all panels stay narrow.

Launches: [stats] [L0] [L1] [L2+pool] [mlp]; between launches the host only
reorders bytes (concatenate / transpose / fancy-index), never does arithmetic
on activations.
"""

import sys

sys.path.insert(0, "/opt/trn_rl_repo")

import contextlib

import numpy as np
import ml_dtypes

import concourse.bacc as bacc
import concourse.bass as bass
import concourse.mybir as mybir
from concourse.tile import TileContext
from concourse.bass_utils import run_bass_kernel_spmd

BF16 = mybir.dt.bfloat16
F32 = mybir.dt.float32
FP8 = mybir.dt.float8e3  # e3m4

NPBF16 = ml_dtypes.bfloat16
NPFP8 = ml_dtypes.float8_e3m4

N_CORES = 8
BN_EPS = 1e-5
NT = 49            # dst tiles per core
NPAD = N_CORES * NT * 128
SHARD = NT * 128
CHUNK_SIZES = [1, 2, 4, 5, 6, 6, 6, 6, 5, 4, 3, 1]

# dtype of the host-expanded per-slot source rows, per layer
DUP_DT = [FP8, FP8, FP8]
DUP_NP = [NPFP8, NPFP8, NPFP8]
# dtype of the h' outputs of layers 0/1 (input precision of the next layer)
OUT_DT = [FP8, FP8]
OUT_NP = [NPFP8, NPFP8]


# ------------------------------------------------------------------ planning
def _plan(src, dst, n_true):
    """Static (h-independent) structure: renumbering, slot stream, panels."""
    degp = np.bincount(dst, minlength=NPAD).astype(np.int64) + 1
    degp[n_true:] = 0

    order = np.argsort(degp, kind="stable")  # new -> orig
    newpos = np.empty(NPAD, np.int64)
    newpos[order] = np.arange(NPAD)          # orig -> new

    # tile k budget: global tile group of 8 (one per core) shares k
    kt = np.zeros(NT, np.int64)
    for t in range(NT):
        kt[t] = degp[order[t * 1024:(t + 1) * 1024]].max()
    kt = np.maximum(kt, 1)

    # block structure per tile: block b covers dst cols [lo, lo+w)
    blocks = []   # per tile: list of (lo, w)
    pan_cols = [] # per tile: list of panel col offsets (into global panel)
    wtot = 0
    for t in range(NT):
        k = int(kt[t])
        bl = []
        for b in range(k):
            lo = (128 * b) // k
            hi = (128 * (b + 1) - 1) // k
            bl.append((lo, hi - lo + 1))
        blocks.append(bl)
        offs = []
        for lo, w in bl:
            offs.append(wtot)
            wtot += w
        pan_cols.append(offs)

    nblk = int(kt.sum())
    tile_base = np.zeros(NT + 1, np.int64)
    tile_base[1:] = np.cumsum(128 * kt)
    meta = {"kt": kt, "blocks": blocks, "pan_cols": pan_cols,
            "wtot": wtot, "nblk": nblk, "order": order, "newpos": newpos,
            "n_true": n_true, "tile_base": tile_base,
            "total_slots": int(tile_base[-1])}
    return meta


def _build_static(meta, src, dst, batch):
    """Per-core constant tables: slot->src map, per-layer panels, rows."""
    kt, blocks, pan_cols = meta["kt"], meta["blocks"], meta["pan_cols"]
    wtot, nblk, order, newpos = (meta["wtot"], meta["nblk"], meta["order"],
                                 meta["newpos"])
    n_true = meta["n_true"]

    deg = np.bincount(dst, minlength=NPAD).astype(np.float64) + 1.0
    dinv = (1.0 / np.sqrt(deg)).astype(np.float64)
    dinv_pad = dinv.copy()
    dinv_pad[n_true:] = 1.0

    # new-indexed per-node values
    dinv_new = dinv_pad[order]
    batch_pad = np.full(NPAD, 0, np.int64)
    batch_pad[:n_true] = batch
    batch_new = batch_pad[order]
    valid_new = (order < n_true)

    # d2[d] = sum over edges of dinv[s]*dinv[d] + dinv[d]^2 (full coef sum)
    sneig = np.bincount(dst, weights=dinv[src], minlength=NPAD)
    d2 = dinv_pad * (sneig + dinv_pad)       # orig indexed
    d2_new = d2[order]

    cnt = np.bincount(batch_pad[:n_true], minlength=64).astype(np.float64)
    invc = (1.0 / np.maximum(cnt, 1.0)).astype(np.float64)

    # per-core slot assignment
    s_new = newpos[src]
    d_new = newpos[dst]
    g_tile = d_new // 128                    # global tile of dst
    core_of = g_tile % N_CORES
    tloc = g_tile // N_CORES
    dloc = d_new % 128

    tile_base = meta["tile_base"]
    total_slots = meta["total_slots"]

    edge_w0 = dinv[src] * dinv_pad[dst] * dinv_pad[dst]   # L0 edge weight*sig

    cores = []
    for c in range(N_CORES):
        sel = core_of == c
        es, et, ed = s_new[sel], tloc[sel], dloc[sel]
        ew0 = edge_w0[sel]
        # order edges of each dst node consecutively
        key = et * (128 * 64) + ed
        o = np.argsort(key, kind="stable")
        es, et, ed, ew0 = es[o], et[o], ed[o], ew0[o]
        # slot position: base + dloc*k + rank within node (self slot first)
        k_of = kt[et]
        node_key = et * 128 + ed
        # rank of edge within its node
        uniq, first_idx, counts = np.unique(node_key, return_index=True,
                                            return_counts=True)
        rank = np.arange(len(node_key)) - np.repeat(first_idx, counts)
        slot = tile_base[et] + ed * k_of + 1 + rank   # +1: self slot at 0

        # own nodes of this core (new index), per (t, d)
        tt = np.arange(NT).repeat(128)
        dd = np.tile(np.arange(128), NT)
        own_new = (tt * N_CORES + np.full(NT * 128, c)) * 128 + dd
        own_valid = valid_new[own_new]
        self_slot = tile_base[tt] + dd * kt[tt]

        slotsrc = np.full(total_slots, NPAD, np.int64)  # NPAD -> zero row
        slotsrc[slot] = es
        slotsrc[self_slot[own_valid]] = own_new[own_valid]

        dv_own = dinv_new[own_new]           # dinv of (c,t,d) node
        # panel weights per slot, per layer
        w_l0 = np.zeros(total_slots, np.float64)
        w_l0[slot] = ew0                                  # dinv[s]*dinv[d]^2
        w_l0[self_slot[own_valid]] = (dv_own ** 3)[own_valid]
        col_dinv = np.repeat(dv_own, np.repeat(kt, 128))  # dinv[d] per slot
        filled = np.zeros(total_slots, bool)
        filled[slot] = True
        filled[self_slot[own_valid]] = True
        w_l1 = np.where(filled, col_dinv ** 2, 0.0)
        w_l2 = np.where(filled, col_dinv, 0.0)

        # panels [128, wtot]
        pans = []
        for wv in (w_l0, w_l1, w_l2):
            pan = np.zeros((128, wtot), np.float64)
            for t in range(NT):
                k = int(kt[t])
                for b, (lo, w) in enumerate(blocks[t]):
                    co = pan_cols[t][b]
                    sl0 = tile_base[t] + b * 128
                    ss = np.arange(sl0, sl0 + 128)
                    cc = (ss - tile_base[t]) // k - lo    # col within panel
                    ok = (cc >= 0) & (cc < w)
                    pan[np.arange(128)[ok], co + cc[ok]] = wv[ss][ok]
            pans.append(pan.astype(NPBF16))

        # packed bf16 rows: sigma_out per col, d2*sigma per col
        sig_row = np.zeros(SHARD, np.float64)
        sh_row = np.zeros(SHARD, np.float64)
        for t in range(NT):
            cols = slice(t * 128, (t + 1) * 128)
            nn = (t * N_CORES + c) * 128 + np.arange(128)
            sig_row[cols] = dinv_new[nn]
            sh_row[cols] = d2_new[nn] * dinv_new[nn]

        # pool panel [128, NT*64]
        gpan = np.zeros((128, NT * 64), np.float64)
        for t in range(NT):
            nn = (t * N_CORES + c) * 128 + np.arange(128)
            gb = batch_new[nn]
            ok = valid_new[nn]
            gpan[np.arange(128)[ok], t * 64 + gb[ok]] = invc[gb[ok]]

        cores.append({
            "slotsrc": slotsrc,
            "pans": pans,
            "sig_row": sig_row,
            "sh_row": sh_row,
            "gpan": gpan.astype(NPBF16),
        })
    return cores


def _dup_layout(h_new, slotsrc, np_dt):
    """[NPAD(+1), F] new-indexed rows -> [128, NBLK*F] slot-stream layout."""
    rows = h_new[slotsrc]                    # [total_slots, F]
    nblk = rows.shape[0] // 128
    F = rows.shape[1]
    return np.ascontiguousarray(
        rows.reshape(nblk, 128, F).transpose(1, 0, 2)
    ).reshape(128, nblk * F).astype(np_dt)


# ------------------------------------------------------------------ programs
def _build_stats_program(meta):
    """Per-core BN partial sums: [128, 2] = (sum x, sum x^2) per feature."""
    F = 128
    nc = bacc.Bacc("TRN2", target_bir_lowering=False, debug=False,
                   num_devices=N_CORES)
    xs_d = nc.dram_tensor("x_sh", [128, NT * F], FP8,
                          kind="ExternalInput").ap()
    ident_d = nc.dram_tensor("ident", [128, 128], F32,
                             kind="ExternalInput").ap()
    out_d = nc.dram_tensor("stat_part", [128, 2], F32,
                           kind="ExternalOutput").ap()
    with TileContext(nc) as tc:
        with tc.tile_pool(name="w", bufs=1) as wp, \
             tc.tile_pool(name="ps", bufs=1, space="PSUM") as pp:
            xs = wp.tile([128, NT * F], FP8, tag="xs")
            QF = 13 * F
            nc.sync.dma_start(out=xs[:, :QF], in_=xs_d[:, :QF])
            ident_s = wp.tile([128, 128], F32, tag="id")
            nc.sync.dma_start(out=ident_s[:], in_=ident_d[:])
            for q in range(1, 4):
                hi = min((13 + q * 12) * F, NT * F)
                nc.sync.dma_start(out=xs[:, hi - 12 * F:hi],
                                  in_=xs_d[:, hi - 12 * F:hi])
            ones_s = wp.tile([128, 1], FP8, tag="ones")
            nc.vector.memset(ones_s[:], 1.0)
            xtx_ps = pp.tile([128, 128], F32, tag="xtx")
            sx_ps = pp.tile([128, 1], F32, tag="sx")
            for t in range(NT):
                sl = xs[:, t * F:(t + 1) * F]
                nc.tensor.matmul(xtx_ps[:], sl, sl, start=(t == 0),
                                 stop=(t == NT - 1))
                nc.tensor.matmul(sx_ps[:], sl, ones_s[:], start=(t == 0),
                                 stop=(t == NT - 1))
            dg = wp.tile([128, 128], F32, tag="dg")
            nc.vector.tensor_tensor(dg[:], xtx_ps[:], ident_s[:],
                                    mybir.AluOpType.mult)
            o = wp.tile([128, 2], F32, tag="o")
            nc.vector.tensor_reduce(o[:, 1:2], dg[:], mybir.AxisListType.X,
                                    mybir.AluOpType.add)
            nc.vector.tensor_copy(o[:, 0:1], sx_ps[:])
            nc.sync.dma_start(out=out_d[:], in_=o[:])
    nc.compile()
    return nc


def _build_layer_program(meta, lay):
    kt, blocks, pan_cols, wtot, nblk, tile_base = (
        meta["kt"], meta["blocks"], meta["pan_cols"], meta["wtot"],
        meta["nblk"], meta["tile_base"])
    F = 128 if lay < 2 else 64     # dup row width (L2 rows pre-transformed)
    H = 128
    H2 = 64
    G = 64
    Ho = H if lay < 2 else H2
    N_true = meta["n_true"]
    dt_in = DUP_DT[lay]
    dt_out = OUT_DT[lay] if lay < 2 else None

    nc = bacc.Bacc("TRN2", target_bir_lowering=False, debug=False,
                   num_devices=N_CORES)

    def din(name, shape, dt):
        return nc.dram_tensor(name, list(shape), dt, kind="ExternalInput").ap()

    dup_d = din("dup", [128, nblk * F], dt_in)
    if lay == 0:
        PW_EXTRA = 0               # W1 travels in f32pack
    elif lay == 1:
        PW_EXTRA = H + H2          # W2 | W3
    else:
        PW_EXTRA = NT * G + 128    # gpan | identity
    pan_d = din("pan", [128, wtot + PW_EXTRA], BF16)
    # packed bf16 row constants
    if lay == 0:
        RP = 2 * SHARD + H        # sig | sh | b1
    elif lay == 1:
        RP = SHARD + H            # sig | b2
    else:
        RP = H2                   # b3
    rp_d = din("rowpack", [1, RP], BF16)
    if lay == 0:
        # sxp | exp | gamma | beta | W1(fp32)
        fp_d = din("f32pack", [128, 18 + H], F32)
    if lay == 2:
        pool_out = nc.dram_tensor("pool_part", [H2, G], F32,
                                  kind="ExternalOutput").ap()
    else:
        OW = 128 if lay == 0 else 64   # L1 outputs t2 = h2' @ W3
        h_out = nc.dram_tensor("h_out", [OW, NT * 128], dt_out,
                               kind="ExternalOutput").ap()

    chunk_tiles = []
    t0 = 0
    for cs in CHUNK_SIZES:
        chunk_tiles.append(list(range(t0, min(t0 + cs, NT))))
        t0 += cs

    with TileContext(nc) as tc:
        with contextlib.ExitStack() as ctx:
            cpool = ctx.enter_context(tc.tile_pool(name="const", bufs=1))
            dpool = ctx.enter_context(tc.tile_pool(name="dup", bufs=5))
            ppool = ctx.enter_context(tc.tile_pool(name="pan", bufs=2))

            # first chunk's data first so its transfer leads the queue
            def chunk_loads(tiles):
                ct0, ct1 = tiles[0], tiles[-1] + 1
                b0 = int(tile_base[ct0] // 128)
                b1 = int(tile_base[ct1] // 128)
                dup_sb = dpool.tile([128, (b1 - b0) * F], dt_in, tag="dup")
                nc.sync.dma_start(out=dup_sb[:], in_=dup_d[:, b0 * F:b1 * F])
                return dup_sb, b0

            pend = [chunk_loads(chunk_tiles[0])]
            pan_sb = ppool.tile([128, wtot + PW_EXTRA], BF16, tag="pan")
            PSPLIT = PW_EXTRA + pan_cols[12][0]
            nc.sync.dma_start(out=pan_sb[:, :PSPLIT], in_=pan_d[:, :PSPLIT])

            rp_s = cpool.tile([1, RP], BF16, tag="c_rp")
            nc.sync.dma_start(out=rp_s[:], in_=rp_d[:])
            if lay == 0:
                fp_s = cpool.tile([128, 18 + H], F32, tag="c_fp")
                nc.sync.dma_start(out=fp_s[:], in_=fp_d[:])
            nc.sync.dma_start(out=pan_sb[:, PSPLIT:], in_=pan_d[:, PSPLIT:])
            if lay == 0:
                sig_s = rp_s[0:1, 0:SHARD]
                sh_s = rp_s[0:1, SHARD:2 * SHARD]
                b_s = rp_s[0:1, 2 * SHARD:2 * SHARD + H]
            elif lay == 1:
                sig_s = rp_s[0:1, 0:SHARD]
                b_s = rp_s[0:1, SHARD:SHARD + H]
            else:
                b_s = rp_s[0:1, 0:H2]
            zr_s = cpool.tile([1, 256], BF16, tag="c_zr")
            nc.vector.memset(zr_s[:], 0.0)
            if lay == 0:
                w1f_s = fp_s[:, 18:18 + H]
                w_s = cpool.tile([128, H], BF16, tag="c_wt")
                rw_s = cpool.tile([1, H], BF16, tag="c_rw")
            elif lay == 1:
                w_s = pan_sb[:, 0:H]
                w3_s = pan_sb[:, H:H + H2]
            else:
                gpan_s = pan_sb[:, 0:NT * G]
                id_s = pan_sb[:, NT * G:NT * G + 128]
                ones_s = cpool.tile([1, 256], BF16, tag="c_ones")
                nc.vector.memset(ones_s[:], 1.0)

            # ---- BN statistics (layer 0) -> W~1 and shift row rw
            if lay == 0:
                with tc.tile_pool(name="ps_st", bufs=1, space="PSUM") as pst, \
                     tc.tile_pool(name="st_w", bufs=2) as stw:
                    sxp_s = fp_s[:, 0:8]
                    exp_s = fp_s[:, 8:16]
                    gam_s = fp_s[:, 16:17]
                    bet_s = fp_s[:, 17:18]
                    ex2 = stw.tile([128, 1], F32, tag="v1")
                    nc.vector.tensor_reduce(ex2[:], exp_s,
                                            mybir.AxisListType.X,
                                            mybir.AluOpType.add)
                    sx = stw.tile([128, 1], F32, tag="v0")
                    nc.vector.tensor_reduce(sx[:], sxp_s,
                                            mybir.AxisListType.X,
                                            mybir.AluOpType.add)
                    mu = stw.tile([128, 1], F32, tag="v2")
                    nc.vector.tensor_scalar_mul(mu[:], sx[:], 1.0 / N_true)
                    var = stw.tile([128, 1], F32, tag="v3")
                    nc.vector.tensor_scalar_mul(var[:], ex2[:], 1.0 / N_true)
                    mu2 = stw.tile([128, 1], F32, tag="v4")
                    nc.vector.tensor_tensor(mu2[:], mu[:], mu[:],
                                            mybir.AluOpType.mult)
                    nc.vector.tensor_tensor(var[:], var[:], mu2[:],
                                            mybir.AluOpType.subtract)
                    nc.vector.tensor_scalar_add(var[:], var[:], BN_EPS)
                    rec = stw.tile([128, 1], F32, tag="v5")
                    nc.vector.reciprocal(rec[:], var[:])
                    isd = stw.tile([128, 1], F32, tag="v6")
                    nc.scalar.activation(isd[:], rec[:],
                                         mybir.ActivationFunctionType.Sqrt)
                    a_c = stw.tile([128, 1], F32, tag="v7")
                    nc.vector.tensor_tensor(a_c[:], gam_s, isd[:],
                                            mybir.AluOpType.mult)
                    nc.vector.tensor_scalar_mul(w_s[:], w1f_s, a_c[:])
                    ca = stw.tile([128, 1], F32, tag="v8")
                    nc.vector.tensor_tensor(ca[:], mu[:], a_c[:],
                                            mybir.AluOpType.mult)
                    nc.vector.tensor_tensor(ca[:], bet_s, ca[:],
                                            mybir.AluOpType.subtract)
                    rw_ps = pst.tile([1, H], F32, tag="rw")
                    nc.tensor.matmul(rw_ps[:], ca[:], w1f_s,
                                     start=True, stop=True)
                    nc.scalar.activation(rw_s[:], rw_ps[:],
                                         mybir.ActivationFunctionType.Copy)

            spool = ctx.enter_context(tc.tile_pool(name="stg", bufs=1))
            wpool = ctx.enter_context(tc.tile_pool(name="wk", bufs=4))
            ps_agg = ctx.enter_context(
                tc.tile_pool(name="ps_agg", bufs=3, space="PSUM"))
            if lay < 2:
                ps_out = ctx.enter_context(
                    tc.tile_pool(name="ps_out", bufs=3, space="PSUM"))
            if lay == 1:
                ps_t = ctx.enter_context(
                    tc.tile_pool(name="ps_t", bufs=2, space="PSUM"))
                tstage = spool.tile([64, NT * 128], dt_out, tag="tstg")
            if lay == 2:
                ps_tr = ctx.enter_context(
                    tc.tile_pool(name="ps_tr", bufs=2, space="PSUM"))
                ps_pl = ctx.enter_context(
                    tc.tile_pool(name="ps_pl", bufs=1, space="PSUM"))
                pool_ps = ps_pl.tile([H2, G], F32, tag="pool")

            if lay == 0:
                stage = spool.tile([128, NT * 128], dt_out, tag="stg")
            elif lay == 1:
                stage = spool.tile([128, NT * 128], BF16, tag="stg")

            state = {"use_dve": False}

            def flip():
                state["use_dve"] = not state["use_dve"]
                return state["use_dve"]

            def phase1(pr, dup_sb, b0):
                """agg matmuls (+ L2: bias + relu straight from PSUM)."""
                pw = len(pr) * 128
                rows = H2 if lay == 2 else 128
                agg_ps = ps_agg.tile([rows, pw], F32, tag="agg")
                if lay == 2:
                    # bias rank-1 doubles as the PSUM reset (full-tile write)
                    nc.tensor.matmul(agg_ps[:], b_s, ones_s[0:1, 0:pw],
                                     start=True, stop=False,
                                     skip_group_check=True)
                else:
                    nc.tensor.matmul(agg_ps[:], zr_s[0:1, 0:rows],
                                     zr_s[0:1, 0:pw], start=True, stop=False,
                                     skip_group_check=True)
                nb_pair = sum(int(kt[t]) for t in pr)
                bi = 0
                for hi, t in enumerate(pr):
                    for b, (lo, w) in enumerate(blocks[t]):
                        gb = int(tile_base[t] // 128) + b
                        co = pan_cols[t][b]
                        bi += 1
                        nc.tensor.matmul(
                            agg_ps[:, hi * 128 + lo:hi * 128 + lo + w],
                            dup_sb[:, (gb - b0) * F:(gb - b0 + 1) * F],
                            pan_sb[:, PW_EXTRA + co:PW_EXTRA + co + w],
                            start=False,
                            stop=(bi == nb_pair),
                            skip_group_check=True)
                if lay == 2:
                    hsT = wpool.tile([H2, pw], BF16, tag="hsT")
                    if flip():
                        nc.vector.tensor_scalar_max(hsT[:], agg_ps[:], 0.0)
                    else:
                        nc.scalar.activation(
                            hsT[:], agg_ps[:],
                            mybir.ActivationFunctionType.Relu)
                    return pr, agg_ps, hsT
                aggT = wpool.tile([128, pw], BF16, tag="aggT")
                if flip():
                    nc.vector.tensor_copy(aggT[:], agg_ps[:])
                else:
                    nc.scalar.activation(aggT[:], agg_ps[:],
                                         mybir.ActivationFunctionType.Copy)
                return pr, agg_ps, aggT

            def phase2(st1):
                pr, agg_ps, aggT = st1
                pw = len(pr) * 128
                if lay < 2:
                    h_ps = ps_out.tile([Ho, pw], F32, tag="hps")
                    for hi, t in enumerate(pr):
                        hsl = slice(hi * 128, (hi + 1) * 128)
                        nc.tensor.matmul(h_ps[:, hsl], w_s[:] if lay == 0
                                         else w_s, aggT[:, hsl],
                                         start=True, stop=False,
                                         skip_group_check=True)
                        nc.tensor.matmul(
                            h_ps[:, hsl], b_s,
                            sig_s[0:1, t * 128:(t + 1) * 128],
                            start=False, stop=(lay != 0),
                            skip_group_check=True)
                        if lay == 0:
                            nc.tensor.matmul(
                                h_ps[:, hsl], rw_s[:],
                                sh_s[0:1, t * 128:(t + 1) * 128],
                                start=False, stop=True,
                                skip_group_check=True)
                    so = pr[0] * 128
                    if flip():
                        nc.vector.tensor_scalar_max(
                            stage[:, so:so + pw], h_ps[:], 0.0)
                    else:
                        nc.scalar.activation(
                            stage[:, so:so + pw], h_ps[:],
                            mybir.ActivationFunctionType.Relu)
                    return st1
                # lay 2: transpose each tile's hsT half: [64, 128] -> [128, 64]
                hsT = aggT
                trs = []
                for hi, t in enumerate(pr):
                    tr_ps = ps_tr.tile([128, H2], BF16, tag="tr")
                    nc.tensor.transpose(tr_ps[:],
                                        hsT[:, hi * 128:(hi + 1) * 128],
                                        id_s[0:64, 0:64])
                    hs_sb = wpool.tile([128, H2], BF16, tag="hs")
                    if flip():
                        nc.vector.tensor_copy(hs_sb[:], tr_ps[:])
                    else:
                        nc.scalar.activation(
                            hs_sb[:], tr_ps[:],
                            mybir.ActivationFunctionType.Copy)
                    trs.append((t, hs_sb))
                return trs

            def phase3(st2):
                if lay == 1:
                    pr = st2[0]
                    pw = len(pr) * 128
                    so = pr[0] * 128
                    t_ps = ps_t.tile([H2, pw], F32, tag="tps")
                    nc.tensor.matmul(t_ps[:], w3_s, stage[:, so:so + pw],
                                     start=True, stop=True,
                                     skip_group_check=True)
                    if flip():
                        nc.vector.tensor_copy(tstage[:, so:so + pw], t_ps[:])
                    else:
                        nc.scalar.activation(
                            tstage[:, so:so + pw], t_ps[:],
                            mybir.ActivationFunctionType.Copy)
                elif lay == 2:
                    for t, hs_sb in st2:
                        nc.tensor.matmul(pool_ps[:], hs_sb[:],
                                         gpan_s[:, t * G:(t + 1) * G],
                                         start=(t == 0), stop=(t == NT - 1),
                                         skip_group_check=True)

            # pair pipeline across all chunks: phase1(i) | phase2(i-1) |
            # phase3(i-2) keeps PE from stalling on Act/DVE results
            all_pairs = []
            for ci, tiles in enumerate(chunk_tiles):
                dup_sb, b0 = pend.pop(0)
                if ci + 1 < len(chunk_tiles):
                    pend.append(chunk_loads(chunk_tiles[ci + 1]))
                for i in range(0, len(tiles), 2):
                    all_pairs.append((tiles[i:i + 2], dup_sb, b0))

            hastail = lay > 0
            q2, q3 = [], []
            out_stage = stage if lay == 0 else (tstage if lay == 1 else None)
            OW = 128 if lay == 0 else 64
            WRITES = [(5, 0, 24), (8, 24, 41), (11, 41, NT)]

            def maybe_write(done_tile):
                if lay == 2:
                    return
                while WRITES and done_tile >= WRITES[0][2] - 1:
                    _, wt0, wt1 = WRITES.pop(0)
                    nc.scalar.dma_start(
                        out=h_out[:, wt0 * 128:wt1 * 128],
                        in_=out_stage[:, wt0 * 128:wt1 * 128])

            for item in all_pairs:
                st1 = phase1(*item)
                if q2:
                    st2 = phase2(q2.pop(0))
                    if hastail:
                        q3.append(st2)
                    else:
                        maybe_write(st2[0][-1])
                if q3:
                    st3 = q3.pop(0)
                    phase3(st3)
                    done = st3[0][-1] if lay == 1 else st3[-1][0]
                    maybe_write(done)
                q2.append(st1)
            while q2 or q3:
                if q2:
                    st2 = phase2(q2.pop(0))
                    if hastail:
                        q3.append(st2)
                    else:
                        maybe_write(st2[0][-1])
                if q3:
                    st3 = q3.pop(0)
                    phase3(st3)
                    done = st3[0][-1] if lay == 1 else st3[-1][0]
                    maybe_write(done)
            if lay == 2:
                po = wpool.tile([H2, G], F32, tag="po")
                nc.vector.tensor_copy(po[:], pool_ps[:])
                nc.sync.dma_start(out=pool_out[:], in_=po[:])

    nc.compile()
    return nc


def _build_mlp_program(meta):
    G, H2, H4, C = 64, 64, 32, 2
    nc = bacc.Bacc("TRN2", target_bir_lowering=False, debug=False,
                   num_devices=N_CORES)
    # pack1 [128, 4G + H2]: cols 0:4G = pool partials (2i in rows 0:64,
    # 2i+1 in rows 64:128), cols 4G: = stacked identity
    pk1_d = nc.dram_tensor("pack1", [128, 4 * G + H2], F32,
                           kind="ExternalInput").ap()
    # pack2 [64, 37]: wc1 | bc2b | wc2 | bc1
    pk2_d = nc.dram_tensor("pack2", [64, 37], F32,
                           kind="ExternalInput").ap()
    out_d = nc.dram_tensor("out", [G, C], F32, kind="ExternalOutput").ap()

    with TileContext(nc) as tc:
        with tc.tile_pool(name="w", bufs=1) as wp, \
             tc.tile_pool(name="ps", bufs=1, space="PSUM") as pp:
            pk1_s = wp.tile([128, 4 * G + H2], F32, tag="pk1")
            nc.sync.dma_start(out=pk1_s[:], in_=pk1_d[:])
            pk2_s = wp.tile([64, 37], F32, tag="pk2")
            nc.sync.dma_start(out=pk2_s[:], in_=pk2_d[:])
            eye2_s = pk1_s[:, 4 * G:4 * G + H2]
            wc1_s = pk2_s[:, 0:32]
            bc2_s = pk2_s[:, 32:34]
            wc2_s = pk2_s[0:32, 34:36]
            bc1_s = pk2_s[0:32, 36:37]

            acc_ps = pp.tile([H2, G], F32, tag="acc")
            for i in range(4):
                nc.tensor.matmul(acc_ps[:], eye2_s,
                                 pk1_s[:, i * G:(i + 1) * G],
                                 start=(i == 0), stop=(i == 3))
            acc_s = wp.tile([H2, G], F32, tag="accs")
            nc.vector.tensor_copy(acc_s[:], acc_ps[:])
            z_ps = pp.tile([H4, G], F32, tag="z")
            nc.tensor.matmul(z_ps[:], wc1_s, acc_s[:], start=True,
                             stop=True)
            z_s = wp.tile([H4, G], F32, tag="zs")
            nc.vector.tensor_scalar(z_s[:], z_ps[:], bc1_s, 0.0,
                                    mybir.AluOpType.add,
                                    mybir.AluOpType.max)
            o_ps = pp.tile([G, C], F32, tag="o")
            nc.tensor.matmul(o_ps[:], z_s[:], wc2_s, start=True, stop=True)
            o_s = wp.tile([G, C], F32, tag="os")
            nc.vector.tensor_tensor(o_s[:], o_ps[:], bc2_s,
                                    mybir.AluOpType.add)
            nc.sync.dma_start(out=out_d[:], in_=o_s[:])
    nc.compile()
    return nc


# ------------------------------------------------------------------ driver
_CACHE = {}


def _get_programs(meta):
    key = (tuple(meta["kt"]), meta["n_true"])
    if key not in _CACHE:
        progs = [_build_stats_program(meta)]
        progs += [_build_layer_program(meta, lay) for lay in range(3)]
        progs.append(_build_mlp_program(meta))
        _CACHE[key] = progs
    return _CACHE[key]


def run_gnn(runner=None, **inputs):
    F, H, H2, H4, C, G = 128, 128, 64, 32, 2, 64
    x = np.asarray(inputs["x"], np.float32)
    n_true = x.shape[0]
    src = np.asarray(inputs["edge_index"][0], np.int64)
    dst = np.asarray(inputs["edge_index"][1], np.int64)
    batch = np.asarray(inputs["batch"], np.int64)

    meta = _plan(src, dst, n_true)
    cores = _build_static(meta, src, dst, batch)
    order = meta["order"]
    progs = _get_programs(meta)

    def run(nc, in_maps):
        if runner is not None:
            return runner(nc, in_maps)
        return run_bass_kernel_spmd(
            nc, in_maps, core_ids=list(range(N_CORES))).results

    # x rows in new order, padded, with an extra zero row at index NPAD
    x_new = np.zeros((NPAD + 1, F), np.float32)
    x_new[:NPAD][order < n_true] = x[order[order < n_true]]

    # ---- stats launch (reads new-order x shards, tile-major per core)
    xb = x_new[:NPAD].astype(NPFP8)
    stats_maps = []
    for c in range(N_CORES):
        idx = ((np.arange(NT) * N_CORES + c)[:, None] * 128
               + np.arange(128)[None, :])          # [NT, 128] node ids
        slab = xb[idx]                             # [NT, 128, F]
        slab = np.ascontiguousarray(slab.transpose(1, 0, 2)).reshape(
            128, NT * F)
        stats_maps.append({"x_sh": slab,
                           "ident": np.eye(128, dtype=np.float32)})
    res = run(progs[0], stats_maps)
    parts = np.stack([np.asarray(res[c]["stat_part"])
                      for c in range(N_CORES)], axis=2)
    sx_parts = np.ascontiguousarray(parts[:, 0, :], dtype=np.float32)
    ex2_parts = np.ascontiguousarray(parts[:, 1, :], dtype=np.float32)

    W = [np.asarray(inputs["W1"], np.float32),
         np.asarray(inputs["W2"], np.float32),
         np.asarray(inputs["W3"], np.float32)]
    brows = [np.asarray(inputs["b1"], np.float32).reshape(1, H),
             np.asarray(inputs["b2"], np.float32).reshape(1, H),
             np.asarray(inputs["b3"], np.float32).reshape(1, H2)]

    h_new = x_new
    pool_parts = None
    for lay in range(3):
        maps = []
        for c in range(N_CORES):
            st = cores[c]
            if lay == 0:
                rp = np.concatenate([st["sig_row"], st["sh_row"],
                                     brows[0].ravel()])
            elif lay == 1:
                rp = np.concatenate([st["sig_row"], brows[1].ravel()])
            else:
                rp = brows[2].ravel()
            pre = []
            if lay == 1:
                pre.append(W[1].astype(NPBF16))
                pre.append(W[2].astype(NPBF16))
            if lay == 2:
                pre.append(st["gpan"])
                pre.append(np.eye(128, dtype=NPBF16))
            pan = np.concatenate(pre + [st["pans"][lay]], axis=1) \
                if pre else st["pans"][lay]
            m = {"dup": _dup_layout(h_new, st["slotsrc"], DUP_NP[lay]),
                 "pan": np.ascontiguousarray(pan),
                 "rowpack": rp.astype(NPBF16).reshape(1, -1)}
            if lay == 0:
                fp = np.zeros((128, 18 + H), np.float32)
                fp[:, 0:8] = sx_parts
                fp[:, 8:16] = ex2_parts
                fp[:, 16] = np.asarray(inputs["bn_gamma"], np.float32)
                fp[:, 17] = np.asarray(inputs["bn_beta"], np.float32)
                fp[:, 18:] = W[0]
                m["f32pack"] = fp
            maps.append(m)
        res = run(progs[1 + lay], maps)
        if lay < 2:
            # h_out [OW, NT*128] per core -> h_new [NPAD+1, OW]
            OW = 128 if lay == 0 else 64
            h_new = np.zeros((NPAD + 1, OW), np.float32)
            for c in range(N_CORES):
                ho = np.asarray(res[c]["h_out"])   # [OW, NT*128]
                hoT = ho.reshape(OW, NT, 128).transpose(1, 2, 0)
                idx = ((np.arange(NT) * N_CORES + c)[:, None] * 128
                       + np.arange(128)[None, :])
                h_new[idx] = hoT
        else:
            pool_parts = [np.asarray(res[c]["pool_part"])
                          for c in range(N_CORES)]

    # ---- MLP launch
    pk1 = np.zeros((128, 4 * G + H2), np.float32)
    for i in range(4):
        pk1[0:H2, i * G:(i + 1) * G] = pool_parts[2 * i]
        pk1[H2:128, i * G:(i + 1) * G] = pool_parts[2 * i + 1]
    pk1[0:H2, 4 * G:] = np.eye(H2, dtype=np.float32)
    pk1[H2:128, 4 * G:] = np.eye(H2, dtype=np.float32)
    pk2 = np.zeros((64, 37), np.float32)
    pk2[:, 0:32] = np.asarray(inputs["Wc1"], np.float32)
    pk2[:, 32:34] = np.tile(np.asarray(inputs["bc2"], np.float32)[None, :],
                            (G, 1))
    pk2[0:32, 34:36] = np.asarray(inputs["Wc2"], np.float32)
    pk2[0:32, 36] = np.asarray(inputs["bc1"], np.float32)
    mlp_map = {"pack1": pk1, "pack2": pk2}
    res = run(progs[4], [dict(mlp_map) for _ in range(N_CORES)])
    return np.asarray(res[0]["out"], np.float32)


def kernel(**inputs):
    return run_gnn(**inputs)


# revision 25
# speedup vs baseline: 1.0049x; 1.0049x over previous
"""Trainium2 Bass kernel for AudioOnlyGNN (3-layer GCN + BatchNorm + mean-pool + MLP).

Structure (v2 — "static slot stream" design):

Nodes are renumbered by degree (host-side, pure index manipulation) and dealt
round-robin to the 8 cores in 128-row tiles, so that every local tile t holds
nodes of near-identical in-degree.  Each tile gets a uniform per-node slot
budget k_t = max in-degree(+self) over that tile across all cores, giving a
*static* slot stream of 128*k_t slots per tile (identical shape on every
core).  For each layer, the host materialises the edge-source rows in slot
order (a pure gather / data movement step, like the baseline's inter-launch
tile_major permutation) so the device reads them with large contiguous DMA
descriptors instead of per-edge gather descriptors.

On device, a 128-slot block contributes to a [F, 128] PSUM tile via a single
matmul whose moving operand is a small static "panel" matrix (slot -> dst
column weight, the GCN normalisation coefficients baked in by the host from
the graph structure).  The per-tile aggregate is then transformed
(W^T @ agg -> [H, dst]) with bias/BN-shift added as rank-1 matmuls, ReLU'd,
and written back.  Layers 0/1 write h'[dst] = dinv[dst]*ReLU(...) (folded
into the panel weights of the next layer), so panels never depend on h.
Tiles are processed in pairs sharing [128, 256] PSUM tiles so the
PSUM->SBUF copies and ReLUs are batched; the PSUM reset is one matmul
against a zero row, which lets all panels stay narrow.

Launches: [stats] [L0] [L1] [L2+pool] [mlp]; between launches the host only
reorders bytes (concatenate / transpose / fancy-index), never does arithmetic
on activations.
"""

import sys

sys.path.insert(0, "/opt/trn_rl_repo")

import contextlib

import numpy as np
import ml_dtypes

import concourse.bacc as bacc
import concourse.bass as bass
import concourse.mybir as mybir
from concourse.tile import TileContext
from concourse.bass_utils import run_bass_kernel_spmd

BF16 = mybir.dt.bfloat16
F32 = mybir.dt.float32
FP8 = mybir.dt.float8e3  # e3m4

NPBF16 = ml_dtypes.bfloat16
NPFP8 = ml_dtypes.float8_e3m4

N_CORES = 8
BN_EPS = 1e-5
NT = 49            # dst tiles per core
NPAD = N_CORES * NT * 128
SHARD = NT * 128
CHUNK_SIZES = [1, 2, 4, 5, 6, 6, 6, 6, 5, 4, 3, 1]

# dtype of the host-expanded per-slot source rows, per layer
DUP_DT = [FP8, FP8, FP8]
DUP_NP = [NPFP8, NPFP8, NPFP8]
# dtype of the h' outputs of layers 0/1 (input precision of the next layer)
OUT_DT = [FP8, FP8]
OUT_NP = [NPFP8, NPFP8]


# ------------------------------------------------------------------ planning
def _plan(src, dst, n_true):
    """Static (h-independent) structure: renumbering, slot stream, panels."""
    degp = np.bincount(dst, minlength=NPAD).astype(np.int64) + 1
    degp[n_true:] = 0

    order = np.argsort(degp, kind="stable")  # new -> orig
    newpos = np.empty(NPAD, np.int64)
    newpos[order] = np.arange(NPAD)          # orig -> new

    # tile k budget: global tile group of 8 (one per core) shares k
    kt = np.zeros(NT, np.int64)
    for t in range(NT):
        kt[t] = degp[order[t * 1024:(t + 1) * 1024]].max()
    kt = np.maximum(kt, 1)

    # block structure per tile: block b covers dst cols [lo, lo+w)
    blocks = []   # per tile: list of (lo, w)
    pan_cols = [] # per tile: list of panel col offsets (into global panel)
    wtot = 0
    for t in range(NT):
        k = int(kt[t])
        bl = []
        for b in range(k):
            lo = (128 * b) // k
            hi = (128 * (b + 1) - 1) // k
            bl.append((lo, hi - lo + 1))
        blocks.append(bl)
        offs = []
        for lo, w in bl:
            offs.append(wtot)
            wtot += w
        pan_cols.append(offs)

    nblk = int(kt.sum())
    tile_base = np.zeros(NT + 1, np.int64)
    tile_base[1:] = np.cumsum(128 * kt)
    meta = {"kt": kt, "blocks": blocks, "pan_cols": pan_cols,
            "wtot": wtot, "nblk": nblk, "order": order, "newpos": newpos,
            "n_true": n_true, "tile_base": tile_base,
            "total_slots": int(tile_base[-1])}
    return meta


def _build_static(meta, src, dst, batch):
    """Per-core constant tables: slot->src map, per-layer panels, rows."""
    kt, blocks, pan_cols = meta["kt"], meta["blocks"], meta["pan_cols"]
    wtot, nblk, order, newpos = (meta["wtot"], meta["nblk"], meta["order"],
                                 meta["newpos"])
    n_true = meta["n_true"]

    deg = np.bincount(dst, minlength=NPAD).astype(np.float64) + 1.0
    dinv = (1.0 / np.sqrt(deg)).astype(np.float64)
    dinv_pad = dinv.copy()
    dinv_pad[n_true:] = 1.0

    # new-indexed per-node values
    dinv_new = dinv_pad[order]
    batch_pad = np.full(NPAD, 0, np.int64)
    batch_pad[:n_true] = batch
    batch_new = batch_pad[order]
    valid_new = (order < n_true)

    # d2[d] = sum over edges of dinv[s]*dinv[d] + dinv[d]^2 (full coef sum)
    sneig = np.bincount(dst, weights=dinv[src], minlength=NPAD)
    d2 = dinv_pad * (sneig + dinv_pad)       # orig indexed
    d2_new = d2[order]

    cnt = np.bincount(batch_pad[:n_true], minlength=64).astype(np.float64)
    invc = (1.0 / np.maximum(cnt, 1.0)).astype(np.float64)

    # per-core slot assignment
    s_new = newpos[src]
    d_new = newpos[dst]
    g_tile = d_new // 128                    # global tile of dst
    core_of = g_tile % N_CORES
    tloc = g_tile // N_CORES
    dloc = d_new % 128

    tile_base = meta["tile_base"]
    total_slots = meta["total_slots"]

    edge_w0 = dinv[src] * dinv_pad[dst] * dinv_pad[dst]   # L0 edge weight*sig

    cores = []
    for c in range(N_CORES):
        sel = core_of == c
        es, et, ed = s_new[sel], tloc[sel], dloc[sel]
        ew0 = edge_w0[sel]
        # order edges of each dst node consecutively
        key = et * (128 * 64) + ed
        o = np.argsort(key, kind="stable")
        es, et, ed, ew0 = es[o], et[o], ed[o], ew0[o]
        # slot position: base + dloc*k + rank within node (self slot first)
        k_of = kt[et]
        node_key = et * 128 + ed
        # rank of edge within its node
        uniq, first_idx, counts = np.unique(node_key, return_index=True,
                                            return_counts=True)
        rank = np.arange(len(node_key)) - np.repeat(first_idx, counts)
        slot = tile_base[et] + ed * k_of + 1 + rank   # +1: self slot at 0

        # own nodes of this core (new index), per (t, d)
        tt = np.arange(NT).repeat(128)
        dd = np.tile(np.arange(128), NT)
        own_new = (tt * N_CORES + np.full(NT * 128, c)) * 128 + dd
        own_valid = valid_new[own_new]
        self_slot = tile_base[tt] + dd * kt[tt]

        slotsrc = np.full(total_slots, NPAD, np.int64)  # NPAD -> zero row
        slotsrc[slot] = es
        slotsrc[self_slot[own_valid]] = own_new[own_valid]

        dv_own = dinv_new[own_new]           # dinv of (c,t,d) node
        # panel weights per slot, per layer
        w_l0 = np.zeros(total_slots, np.float64)
        w_l0[slot] = ew0                                  # dinv[s]*dinv[d]^2
        w_l0[self_slot[own_valid]] = (dv_own ** 3)[own_valid]
        col_dinv = np.repeat(dv_own, np.repeat(kt, 128))  # dinv[d] per slot
        filled = np.zeros(total_slots, bool)
        filled[slot] = True
        filled[self_slot[own_valid]] = True
        w_l1 = np.where(filled, col_dinv ** 2, 0.0)
        w_l2 = np.where(filled, col_dinv, 0.0)

        # panels [128, wtot]
        pans = []
        for wv in (w_l0, w_l1, w_l2):
            pan = np.zeros((128, wtot), np.float64)
            for t in range(NT):
                k = int(kt[t])
                for b, (lo, w) in enumerate(blocks[t]):
                    co = pan_cols[t][b]
                    sl0 = tile_base[t] + b * 128
                    ss = np.arange(sl0, sl0 + 128)
                    cc = (ss - tile_base[t]) // k - lo    # col within panel
                    ok = (cc >= 0) & (cc < w)
                    pan[np.arange(128)[ok], co + cc[ok]] = wv[ss][ok]
            pans.append(pan.astype(NPBF16))

        # packed bf16 rows: sigma_out per col, d2*sigma per col
        sig_row = np.zeros(SHARD, np.float64)
        sh_row = np.zeros(SHARD, np.float64)
        for t in range(NT):
            cols = slice(t * 128, (t + 1) * 128)
            nn = (t * N_CORES + c) * 128 + np.arange(128)
            sig_row[cols] = dinv_new[nn]
            sh_row[cols] = d2_new[nn] * dinv_new[nn]

        # pool panel [128, NT*64]
        gpan = np.zeros((128, NT * 64), np.float64)
        for t in range(NT):
            nn = (t * N_CORES + c) * 128 + np.arange(128)
            gb = batch_new[nn]
            ok = valid_new[nn]
            gpan[np.arange(128)[ok], t * 64 + gb[ok]] = invc[gb[ok]]

        cores.append({
            "slotsrc": slotsrc,
            "pans": pans,
            "sig_row": sig_row,
            "sh_row": sh_row,
            "gpan": gpan.astype(NPBF16),
        })
    return cores


def _dup_layout(h_new, slotsrc, np_dt):
    """[NPAD(+1), F] new-indexed rows -> [128, NBLK*F] slot-stream layout."""
    rows = h_new[slotsrc]                    # [total_slots, F]
    nblk = rows.shape[0] // 128
    F = rows.shape[1]
    return np.ascontiguousarray(
        rows.reshape(nblk, 128, F).transpose(1, 0, 2)
    ).reshape(128, nblk * F).astype(np_dt)


# ------------------------------------------------------------------ programs
def _build_stats_program(meta):
    """Per-core BN partial sums: [128, 2] = (sum x, sum x^2) per feature."""
    F = 128
    nc = bacc.Bacc("TRN2", target_bir_lowering=False, debug=False,
                   num_devices=N_CORES)
    xs_d = nc.dram_tensor("x_sh", [128, NT * F], FP8,
                          kind="ExternalInput").ap()
    ident_d = nc.dram_tensor("ident", [128, 128], F32,
                             kind="ExternalInput").ap()
    out_d = nc.dram_tensor("stat_part", [128, 2], F32,
                           kind="ExternalOutput").ap()
    with TileContext(nc) as tc:
        with tc.tile_pool(name="w", bufs=1) as wp, \
             tc.tile_pool(name="ps", bufs=1, space="PSUM") as pp:
            xs = wp.tile([128, NT * F], FP8, tag="xs")
            QF = 13 * F
            nc.sync.dma_start(out=xs[:, :QF], in_=xs_d[:, :QF])
            ident_s = wp.tile([128, 128], F32, tag="id")
            nc.sync.dma_start(out=ident_s[:], in_=ident_d[:])
            for q in range(1, 4):
                hi = min((13 + q * 12) * F, NT * F)
                nc.sync.dma_start(out=xs[:, hi - 12 * F:hi],
                                  in_=xs_d[:, hi - 12 * F:hi])
            ones_s = wp.tile([128, 1], FP8, tag="ones")
            nc.vector.memset(ones_s[:], 1.0)
            xtx_ps = pp.tile([128, 128], F32, tag="xtx")
            sx_ps = pp.tile([128, 1], F32, tag="sx")
            for t in range(NT):
                sl = xs[:, t * F:(t + 1) * F]
                nc.tensor.matmul(xtx_ps[:], sl, sl, start=(t == 0),
                                 stop=(t == NT - 1))
                nc.tensor.matmul(sx_ps[:], sl, ones_s[:], start=(t == 0),
                                 stop=(t == NT - 1))
            dg = wp.tile([128, 128], F32, tag="dg")
            nc.vector.tensor_tensor(dg[:], xtx_ps[:], ident_s[:],
                                    mybir.AluOpType.mult)
            o = wp.tile([128, 2], F32, tag="o")
            nc.vector.tensor_reduce(o[:, 1:2], dg[:], mybir.AxisListType.X,
                                    mybir.AluOpType.add)
            nc.vector.tensor_copy(o[:, 0:1], sx_ps[:])
            nc.sync.dma_start(out=out_d[:], in_=o[:])
    nc.compile()
    return nc


def _build_layer_program(meta, lay):
    kt, blocks, pan_cols, wtot, nblk, tile_base = (
        meta["kt"], meta["blocks"], meta["pan_cols"], meta["wtot"],
        meta["nblk"], meta["tile_base"])
    F = 128 if lay < 2 else 64     # dup row width (L2 rows pre-transformed)
    H = 128
    H2 = 64
    G = 64
    Ho = H if lay < 2 else H2
    N_true = meta["n_true"]
    dt_in = DUP_DT[lay]
    dt_out = OUT_DT[lay] if lay < 2 else None

    nc = bacc.Bacc("TRN2", target_bir_lowering=False, debug=False,
                   num_devices=N_CORES)

    def din(name, shape, dt):
        return nc.dram_tensor(name, list(shape), dt, kind="ExternalInput").ap()

    dup_d = din("dup", [128, nblk * F], dt_in)
    if lay == 0:
        PW_EXTRA = 0               # W1 travels in f32pack
    elif lay == 1:
        PW_EXTRA = H + H2          # W2 | W3
    else:
        PW_EXTRA = NT * G + 128    # gpan | identity
    pan_d = din("pan", [128, wtot + PW_EXTRA], BF16)
    # packed bf16 row constants
    if lay == 0:
        RP = 2 * SHARD + H        # sig | sh | b1
    elif lay == 1:
        RP = SHARD + H            # sig | b2
    else:
        RP = H2                   # b3
    rp_d = din("rowpack", [1, RP], BF16)
    if lay == 0:
        # sxp | exp | gamma | beta | W1(fp32)
        fp_d = din("f32pack", [128, 18 + H], F32)
    if lay == 2:
        pool_out = nc.dram_tensor("pool_part", [H2, G], F32,
                                  kind="ExternalOutput").ap()
    else:
        OW = 128 if lay == 0 else 64   # L1 outputs t2 = h2' @ W3
        h_out = nc.dram_tensor("h_out", [OW, NT * 128], dt_out,
                               kind="ExternalOutput").ap()

    chunk_tiles = []
    t0 = 0
    for cs in CHUNK_SIZES:
        chunk_tiles.append(list(range(t0, min(t0 + cs, NT))))
        t0 += cs

    with TileContext(nc) as tc:
        with contextlib.ExitStack() as ctx:
            cpool = ctx.enter_context(tc.tile_pool(name="const", bufs=1))
            dpool = ctx.enter_context(tc.tile_pool(name="dup", bufs=5))
            ppool = ctx.enter_context(tc.tile_pool(name="pan", bufs=2))

            # first chunk's data first so its transfer leads the queue
            def chunk_loads(tiles):
                ct0, ct1 = tiles[0], tiles[-1] + 1
                b0 = int(tile_base[ct0] // 128)
                b1 = int(tile_base[ct1] // 128)
                dup_sb = dpool.tile([128, (b1 - b0) * F], dt_in, tag="dup")
                nc.sync.dma_start(out=dup_sb[:], in_=dup_d[:, b0 * F:b1 * F])
                return dup_sb, b0

            pend = [chunk_loads(chunk_tiles[0])]
            pan_sb = ppool.tile([128, wtot + PW_EXTRA], BF16, tag="pan")
            PSPLIT = PW_EXTRA + pan_cols[12][0]
            nc.sync.dma_start(out=pan_sb[:, :PSPLIT], in_=pan_d[:, :PSPLIT])

            rp_s = cpool.tile([1, RP], BF16, tag="c_rp")
            nc.sync.dma_start(out=rp_s[:], in_=rp_d[:])
            if lay == 0:
                fp_s = cpool.tile([128, 18 + H], F32, tag="c_fp")
                nc.sync.dma_start(out=fp_s[:], in_=fp_d[:])
            nc.sync.dma_start(out=pan_sb[:, PSPLIT:], in_=pan_d[:, PSPLIT:])
            if lay == 0:
                sig_s = rp_s[0:1, 0:SHARD]
                sh_s = rp_s[0:1, SHARD:2 * SHARD]
                b_s = rp_s[0:1, 2 * SHARD:2 * SHARD + H]
            elif lay == 1:
                sig_s = rp_s[0:1, 0:SHARD]
                b_s = rp_s[0:1, SHARD:SHARD + H]
            else:
                b_s = rp_s[0:1, 0:H2]
            zr_s = cpool.tile([1, 256], BF16, tag="c_zr")
            nc.vector.memset(zr_s[:], 0.0)
            if lay == 0:
                w1f_s = fp_s[:, 18:18 + H]
                w_s = cpool.tile([128, H], BF16, tag="c_wt")
                rw_s = cpool.tile([1, H], BF16, tag="c_rw")
            elif lay == 1:
                w_s = pan_sb[:, 0:H]
                w3_s = pan_sb[:, H:H + H2]
            else:
                gpan_s = pan_sb[:, 0:NT * G]
                id_s = pan_sb[:, NT * G:NT * G + 128]
                ones_s = cpool.tile([1, 256], BF16, tag="c_ones")
                nc.vector.memset(ones_s[:], 1.0)

            # ---- BN statistics (layer 0) -> W~1 and shift row rw
            if lay == 0:
                with tc.tile_pool(name="ps_st", bufs=1, space="PSUM") as pst, \
                     tc.tile_pool(name="st_w", bufs=2) as stw:
                    sxp_s = fp_s[:, 0:8]
                    exp_s = fp_s[:, 8:16]
                    gam_s = fp_s[:, 16:17]
                    bet_s = fp_s[:, 17:18]
                    ex2 = stw.tile([128, 1], F32, tag="v1")
                    nc.vector.tensor_reduce(ex2[:], exp_s,
                                            mybir.AxisListType.X,
                                            mybir.AluOpType.add)
                    sx = stw.tile([128, 1], F32, tag="v0")
                    nc.vector.tensor_reduce(sx[:], sxp_s,
                                            mybir.AxisListType.X,
                                            mybir.AluOpType.add)
                    mu = stw.tile([128, 1], F32, tag="v2")
                    nc.vector.tensor_scalar_mul(mu[:], sx[:], 1.0 / N_true)
                    var = stw.tile([128, 1], F32, tag="v3")
                    nc.vector.tensor_scalar_mul(var[:], ex2[:], 1.0 / N_true)
                    mu2 = stw.tile([128, 1], F32, tag="v4")
                    nc.vector.tensor_tensor(mu2[:], mu[:], mu[:],
                                            mybir.AluOpType.mult)
                    nc.vector.tensor_tensor(var[:], var[:], mu2[:],
                                            mybir.AluOpType.subtract)
                    nc.vector.tensor_scalar_add(var[:], var[:], BN_EPS)
                    rec = stw.tile([128, 1], F32, tag="v5")
                    nc.vector.reciprocal(rec[:], var[:])
                    isd = stw.tile([128, 1], F32, tag="v6")
                    nc.scalar.activation(isd[:], rec[:],
                                         mybir.ActivationFunctionType.Sqrt)
                    a_c = stw.tile([128, 1], F32, tag="v7")
                    nc.vector.tensor_tensor(a_c[:], gam_s, isd[:],
                                            mybir.AluOpType.mult)
                    nc.vector.tensor_scalar_mul(w_s[:], w1f_s, a_c[:])
                    ca = stw.tile([128, 1], F32, tag="v8")
                    nc.vector.tensor_tensor(ca[:], mu[:], a_c[:],
                                            mybir.AluOpType.mult)
                    nc.vector.tensor_tensor(ca[:], bet_s, ca[:],
                                            mybir.AluOpType.subtract)
                    rw_ps = pst.tile([1, H], F32, tag="rw")
                    nc.tensor.matmul(rw_ps[:], ca[:], w1f_s,
                                     start=True, stop=True)
                    nc.scalar.activation(rw_s[:], rw_ps[:],
                                         mybir.ActivationFunctionType.Copy)

            spool = ctx.enter_context(tc.tile_pool(name="stg", bufs=1))
            wpool = ctx.enter_context(tc.tile_pool(name="wk", bufs=4))
            ps_agg = ctx.enter_context(
                tc.tile_pool(name="ps_agg", bufs=3, space="PSUM"))
            if lay < 2:
                ps_out = ctx.enter_context(
                    tc.tile_pool(name="ps_out", bufs=3, space="PSUM"))
            if lay == 1:
                ps_t = ctx.enter_context(
                    tc.tile_pool(name="ps_t", bufs=2, space="PSUM"))
                tstage = spool.tile([64, NT * 128], dt_out, tag="tstg")
            if lay == 2:
                ps_tr = ctx.enter_context(
                    tc.tile_pool(name="ps_tr", bufs=2, space="PSUM"))
                ps_pl = ctx.enter_context(
                    tc.tile_pool(name="ps_pl", bufs=1, space="PSUM"))
                pool_ps = ps_pl.tile([H2, G], F32, tag="pool")

            if lay == 0:
                stage = spool.tile([128, NT * 128], dt_out, tag="stg")
            elif lay == 1:
                stage = spool.tile([128, NT * 128], BF16, tag="stg")

            state = {"use_dve": False}

            def flip():
                state["use_dve"] = not state["use_dve"]
                return state["use_dve"]

            def phase1(pr, dup_sb, b0):
                """agg matmuls (+ L2: bias + relu straight from PSUM)."""
                pw = len(pr) * 128
                rows = H2 if lay == 2 else 128
                agg_ps = ps_agg.tile([rows, pw], F32, tag="agg")
                nc.tensor.matmul(agg_ps[:], zr_s[0:1, 0:rows],
                                 zr_s[0:1, 0:pw], start=True, stop=False,
                                 skip_group_check=True)
                nb_pair = sum(int(kt[t]) for t in pr)
                bi = 0
                for hi, t in enumerate(pr):
                    for b, (lo, w) in enumerate(blocks[t]):
                        gb = int(tile_base[t] // 128) + b
                        co = pan_cols[t][b]
                        bi += 1
                        nc.tensor.matmul(
                            agg_ps[:, hi * 128 + lo:hi * 128 + lo + w],
                            dup_sb[:, (gb - b0) * F:(gb - b0 + 1) * F],
                            pan_sb[:, PW_EXTRA + co:PW_EXTRA + co + w],
                            start=False,
                            stop=(bi == nb_pair and lay != 2),
                            skip_group_check=True)
                if lay == 2:
                    nc.tensor.matmul(agg_ps[:], b_s, ones_s[0:1, 0:pw],
                                     start=False, stop=True,
                                     skip_group_check=True)
                    hsT = wpool.tile([H2, pw], BF16, tag="hsT")
                    if flip():
                        nc.vector.tensor_scalar_max(hsT[:], agg_ps[:], 0.0)
                    else:
                        nc.scalar.activation(
                            hsT[:], agg_ps[:],
                            mybir.ActivationFunctionType.Relu)
                    return pr, agg_ps, hsT
                aggT = wpool.tile([128, pw], BF16, tag="aggT")
                if flip():
                    nc.vector.tensor_copy(aggT[:], agg_ps[:])
                else:
                    nc.scalar.activation(aggT[:], agg_ps[:],
                                         mybir.ActivationFunctionType.Copy)
                return pr, agg_ps, aggT

            def phase2(st1):
                pr, agg_ps, aggT = st1
                pw = len(pr) * 128
                if lay < 2:
                    h_ps = ps_out.tile([Ho, pw], F32, tag="hps")
                    for hi, t in enumerate(pr):
                        hsl = slice(hi * 128, (hi + 1) * 128)
                        nc.tensor.matmul(h_ps[:, hsl], w_s[:] if lay == 0
                                         else w_s, aggT[:, hsl],
                                         start=True, stop=False,
                                         skip_group_check=True)
                        nc.tensor.matmul(
                            h_ps[:, hsl], b_s,
                            sig_s[0:1, t * 128:(t + 1) * 128],
                            start=False, stop=(lay != 0),
                            skip_group_check=True)
                        if lay == 0:
                            nc.tensor.matmul(
                                h_ps[:, hsl], rw_s[:],
                                sh_s[0:1, t * 128:(t + 1) * 128],
                                start=False, stop=True,
                                skip_group_check=True)
                    so = pr[0] * 128
                    if flip():
                        nc.vector.tensor_scalar_max(
                            stage[:, so:so + pw], h_ps[:], 0.0)
                    else:
                        nc.scalar.activation(
                            stage[:, so:so + pw], h_ps[:],
                            mybir.ActivationFunctionType.Relu)
                    return st1
                # lay 2: transpose each tile's hsT half: [64, 128] -> [128, 64]
                hsT = aggT
                trs = []
                for hi, t in enumerate(pr):
                    tr_ps = ps_tr.tile([128, H2], BF16, tag="tr")
                    nc.tensor.transpose(tr_ps[:],
                                        hsT[:, hi * 128:(hi + 1) * 128],
                                        id_s[0:64, 0:64])
                    hs_sb = wpool.tile([128, H2], BF16, tag="hs")
                    if flip():
                        nc.vector.tensor_copy(hs_sb[:], tr_ps[:])
                    else:
                        nc.scalar.activation(
                            hs_sb[:], tr_ps[:],
                            mybir.ActivationFunctionType.Copy)
                    trs.append((t, hs_sb))
                return trs

            def phase3(st2):
                if lay == 1:
                    pr = st2[0]
                    pw = len(pr) * 128
                    so = pr[0] * 128
                    t_ps = ps_t.tile([H2, pw], F32, tag="tps")
                    nc.tensor.matmul(t_ps[:], w3_s, stage[:, so:so + pw],
                                     start=True, stop=True,
                                     skip_group_check=True)
                    if flip():
                        nc.vector.tensor_copy(tstage[:, so:so + pw], t_ps[:])
                    else:
                        nc.scalar.activation(
                            tstage[:, so:so + pw], t_ps[:],
                            mybir.ActivationFunctionType.Copy)
                elif lay == 2:
                    for t, hs_sb in st2:
                        nc.tensor.matmul(pool_ps[:], hs_sb[:],
                                         gpan_s[:, t * G:(t + 1) * G],
                                         start=(t == 0), stop=(t == NT - 1),
                                         skip_group_check=True)

            # pair pipeline across all chunks: phase1(i) | phase2(i-1) |
            # phase3(i-2) keeps PE from stalling on Act/DVE results
            all_pairs = []
            for ci, tiles in enumerate(chunk_tiles):
                dup_sb, b0 = pend.pop(0)
                if ci + 1 < len(chunk_tiles):
                    pend.append(chunk_loads(chunk_tiles[ci + 1]))
                for i in range(0, len(tiles), 2):
                    all_pairs.append((tiles[i:i + 2], dup_sb, b0))

            hastail = lay > 0
            q2, q3 = [], []
            out_stage = stage if lay == 0 else (tstage if lay == 1 else None)
            OW = 128 if lay == 0 else 64
            WRITES = [(5, 0, 24), (8, 24, 41), (11, 41, NT)]

            def maybe_write(done_tile):
                if lay == 2:
                    return
                while WRITES and done_tile >= WRITES[0][2] - 1:
                    _, wt0, wt1 = WRITES.pop(0)
                    nc.scalar.dma_start(
                        out=h_out[:, wt0 * 128:wt1 * 128],
                        in_=out_stage[:, wt0 * 128:wt1 * 128])

            for item in all_pairs:
                st1 = phase1(*item)
                if q2:
                    st2 = phase2(q2.pop(0))
                    if hastail:
                        q3.append(st2)
                    else:
                        maybe_write(st2[0][-1])
                if q3:
                    st3 = q3.pop(0)
                    phase3(st3)
                    done = st3[0][-1] if lay == 1 else st3[-1][0]
                    maybe_write(done)
                q2.append(st1)
            while q2 or q3:
                if q2:
                    st2 = phase2(q2.pop(0))
                    if hastail:
                        q3.append(st2)
                    else:
                        maybe_write(st2[0][-1])
                if q3:
                    st3 = q3.pop(0)
                    phase3(st3)
                    done = st3[0][-1] if lay == 1 else st3[-1][0]
                    maybe_write(done)
            if lay == 2:
                po = wpool.tile([H2, G], F32, tag="po")
                nc.vector.tensor_copy(po[:], pool_ps[:])
                nc.sync.dma_start(out=pool_out[:], in_=po[:])

    nc.compile()
    return nc


def _build_mlp_program(meta):
    G, H2, H4, C = 64, 64, 32, 2
    nc = bacc.Bacc("TRN2", target_bir_lowering=False, debug=False,
                   num_devices=N_CORES)
    # pack1 [128, 4G + H2]: cols 0:4G = pool partials (2i in rows 0:64,
    # 2i+1 in rows 64:128), cols 4G: = stacked identity
    pk1_d = nc.dram_tensor("pack1", [128, 4 * G + H2], F32,
                           kind="ExternalInput").ap()
    # pack2 [64, 37]: wc1 | bc2b | wc2 | bc1
    pk2_d = nc.dram_tensor("pack2", [64, 37], F32,
                           kind="ExternalInput").ap()
    out_d = nc.dram_tensor("out", [G, C], F32, kind="ExternalOutput").ap()

    with TileContext(nc) as tc:
        with tc.tile_pool(name="w", bufs=1) as wp, \
             tc.tile_pool(name="ps", bufs=1, space="PSUM") as pp:
            pk1_s = wp.tile([128, 4 * G + H2], F32, tag="pk1")
            nc.sync.dma_start(out=pk1_s[:], in_=pk1_d[:])
            pk2_s = wp.tile([64, 37], F32, tag="pk2")
            nc.sync.dma_start(out=pk2_s[:], in_=pk2_d[:])
            eye2_s = pk1_s[:, 4 * G:4 * G + H2]
            wc1_s = pk2_s[:, 0:32]
            bc2_s = pk2_s[:, 32:34]
            wc2_s = pk2_s[0:32, 34:36]
            bc1_s = pk2_s[0:32, 36:37]

            acc_ps = pp.tile([H2, G], F32, tag="acc")
            for i in range(4):
                nc.tensor.matmul(acc_ps[:], eye2_s,
                                 pk1_s[:, i * G:(i + 1) * G],
                                 start=(i == 0), stop=(i == 3))
            acc_s = wp.tile([H2, G], F32, tag="accs")
            nc.vector.tensor_copy(acc_s[:], acc_ps[:])
            z_ps = pp.tile([H4, G], F32, tag="z")
            nc.tensor.matmul(z_ps[:], wc1_s, acc_s[:], start=True,
                             stop=True)
            z_s = wp.tile([H4, G], F32, tag="zs")
            nc.vector.tensor_scalar(z_s[:], z_ps[:], bc1_s, 0.0,
                                    mybir.AluOpType.add,
                                    mybir.AluOpType.max)
            o_ps = pp.tile([G, C], F32, tag="o")
            nc.tensor.matmul(o_ps[:], z_s[:], wc2_s, start=True, stop=True)
            o_s = wp.tile([G, C], F32, tag="os")
            nc.vector.tensor_tensor(o_s[:], o_ps[:], bc2_s,
                                    mybir.AluOpType.add)
            nc.sync.dma_start(out=out_d[:], in_=o_s[:])
    nc.compile()
    return nc


# ------------------------------------------------------------------ driver
_CACHE = {}


def _get_programs(meta):
    key = (tuple(meta["kt"]), meta["n_true"])
    if key not in _CACHE:
        progs = [_build_stats_program(meta)]
        progs += [_build_layer_program(meta, lay) for lay in range(3)]
        progs.append(_build_mlp_program(meta))
        _CACHE[key] = progs
    return _CACHE[key]


def run_gnn(runner=None, **inputs):
    F, H, H2, H4, C, G = 128, 128, 64, 32, 2, 64
    x = np.asarray(inputs["x"], np.float32)
    n_true = x.shape[0]
    src = np.asarray(inputs["edge_index"][0], np.int64)
    dst = np.asarray(inputs["edge_index"][1], np.int64)
    batch = np.asarray(inputs["batch"], np.int64)

    meta = _plan(src, dst, n_true)
    cores = _build_static(meta, src, dst, batch)
    order = meta["order"]
    progs = _get_programs(meta)

    def run(nc, in_maps):
        if runner is not None:
            return runner(nc, in_maps)
        return run_bass_kernel_spmd(
            nc, in_maps, core_ids=list(range(N_CORES))).results

    # x rows in new order, padded, with an extra zero row at index NPAD
    x_new = np.zeros((NPAD + 1, F), np.float32)
    x_new[:NPAD][order < n_true] = x[order[order < n_true]]

    # ---- stats launch (reads new-order x shards, tile-major per core)
    xb = x_new[:NPAD].astype(NPFP8)
    stats_maps = []
    for c in range(N_CORES):
        idx = ((np.arange(NT) * N_CORES + c)[:, None] * 128
               + np.arange(128)[None, :])          # [NT, 128] node ids
        slab = xb[idx]                             # [NT, 128, F]
        slab = np.ascontiguousarray(slab.transpose(1, 0, 2)).reshape(
            128, NT * F)
        stats_maps.append({"x_sh": slab,
                           "ident": np.eye(128, dtype=np.float32)})
    res = run(progs[0], stats_maps)
    parts = np.stack([np.asarray(res[c]["stat_part"])
                      for c in range(N_CORES)], axis=2)
    sx_parts = np.ascontiguousarray(parts[:, 0, :], dtype=np.float32)
    ex2_parts = np.ascontiguousarray(parts[:, 1, :], dtype=np.float32)

    W = [np.asarray(inputs["W1"], np.float32),
         np.asarray(inputs["W2"], np.float32),
         np.asarray(inputs["W3"], np.float32)]
    brows = [np.asarray(inputs["b1"], np.float32).reshape(1, H),
             np.asarray(inputs["b2"], np.float32).reshape(1, H),
             np.asarray(inputs["b3"], np.float32).reshape(1, H2)]

    h_new = x_new
    pool_parts = None
    for lay in range(3):
        maps = []
        for c in range(N_CORES):
            st = cores[c]
            if lay == 0:
                rp = np.concatenate([st["sig_row"], st["sh_row"],
                                     brows[0].ravel()])
            elif lay == 1:
                rp = np.concatenate([st["sig_row"], brows[1].ravel()])
            else:
                rp = brows[2].ravel()
            pre = []
            if lay == 1:
                pre.append(W[1].astype(NPBF16))
                pre.append(W[2].astype(NPBF16))
            if lay == 2:
                pre.append(st["gpan"])
                pre.append(np.eye(128, dtype=NPBF16))
            pan = np.concatenate(pre + [st["pans"][lay]], axis=1) \
                if pre else st["pans"][lay]
            m = {"dup": _dup_layout(h_new, st["slotsrc"], DUP_NP[lay]),
                 "pan": np.ascontiguousarray(pan),
                 "rowpack": rp.astype(NPBF16).reshape(1, -1)}
            if lay == 0:
                fp = np.zeros((128, 18 + H), np.float32)
                fp[:, 0:8] = sx_parts
                fp[:, 8:16] = ex2_parts
                fp[:, 16] = np.asarray(inputs["bn_gamma"], np.float32)
                fp[:, 17] = np.asarray(inputs["bn_beta"], np.float32)
                fp[:, 18:] = W[0]
                m["f32pack"] = fp
            maps.append(m)
        res = run(progs[1 + lay], maps)
        if lay < 2:
            # h_out [OW, NT*128] per core -> h_new [NPAD+1, OW]
            OW = 128 if lay == 0 else 64
            h_new = np.zeros((NPAD + 1, OW), np.float32)
            for c in range(N_CORES):
                ho = np.asarray(res[c]["h_out"])   # [OW, NT*128]
                hoT = ho.reshape(OW, NT, 128).transpose(1, 2, 0)
                idx = ((np.arange(NT) * N_CORES + c)[:, None] * 128
                       + np.arange(128)[None, :])
                h_new[idx] = hoT
        else:
            pool_parts = [np.asarray(res[c]["pool_part"])
                          for c in range(N_CORES)]

    # ---- MLP launch
    pk1 = np.zeros((128, 4 * G + H2), np.float32)
    for i in range(4):
        pk1[0:H2, i * G:(i + 1) * G] = pool_parts[2 * i]
        pk1[H2:128, i * G:(i + 1) * G] = pool_parts[2 * i + 1]
    pk1[0:H2, 4 * G:] = np.eye(H2, dtype=np.float32)
    pk1[H2:128, 4 * G:] = np.eye(H2, dtype=np.float32)
    pk2 = np.zeros((64, 37), np.float32)
    pk2[:, 0:32] = np.asarray(inputs["Wc1"], np.float32)
    pk2[:, 32:34] = np.tile(np.asarray(inputs["bc2"], np.float32)[None, :],
                            (G, 1))
    pk2[0:32, 34:36] = np.asarray(inputs["Wc2"], np.float32)
    pk2[0:32, 36] = np.asarray(inputs["bc1"], np.float32)
    mlp_map = {"pack1": pk1, "pack2": pk2}
    res = run(progs[4], [dict(mlp_map) for _ in range(N_CORES)])
    return np.asarray(res[0]["out"], np.float32)


def kernel(**inputs):
    return run_gnn(**inputs)


# revision 26
# speedup vs baseline: 1.0060x; 1.0011x over previous
"""Trainium2 Bass kernel for AudioOnlyGNN (3-layer GCN + BatchNorm + mean-pool + MLP).

Structure (v2 — "static slot stream" design):

Nodes are renumbered by degree (host-side, pure index manipulation) and dealt
round-robin to the 8 cores in 128-row tiles, so that every local tile t holds
nodes of near-identical in-degree.  Each tile gets a uniform per-node slot
budget k_t = max in-degree(+self) over that tile across all cores, giving a
*static* slot stream of 128*k_t slots per tile (identical shape on every
core).  For each layer, the host materialises the edge-source rows in slot
order (a pure gather / data movement step, like the baseline's inter-launch
tile_major permutation) so the device reads them with large contiguous DMA
descriptors instead of per-edge gather descriptors.

On device, a 128-slot block contributes to a [F, 128] PSUM tile via a single
matmul whose moving operand is a small static "panel" matrix (slot -> dst
column weight, the GCN normalisation coefficients baked in by the host from
the graph structure).  The per-tile aggregate is then transformed
(W^T @ agg -> [H, dst]) with bias/BN-shift added as rank-1 matmuls, ReLU'd,
and written back.  Layers 0/1 write h'[dst] = dinv[dst]*ReLU(...) (folded
into the panel weights of the next layer), so panels never depend on h.
Tiles are processed in pairs sharing [128, 256] PSUM tiles so the
PSUM->SBUF copies and ReLUs are batched; the PSUM reset is one matmul
against a zero row, which lets all panels stay narrow.

Launches: [stats] [L0] [L1] [L2+pool] [mlp]; between launches the host only
reorders bytes (concatenate / transpose / fancy-index), never does arithmetic
on activations.
"""

import sys

sys.path.insert(0, "/opt/trn_rl_repo")

import contextlib

import numpy as np
import ml_dtypes

import concourse.bacc as bacc
import concourse.bass as bass
import concourse.mybir as mybir
from concourse.tile import TileContext
from concourse.bass_utils import run_bass_kernel_spmd

BF16 = mybir.dt.bfloat16
F32 = mybir.dt.float32
FP8 = mybir.dt.float8e3  # e3m4

NPBF16 = ml_dtypes.bfloat16
NPFP8 = ml_dtypes.float8_e3m4

N_CORES = 8
BN_EPS = 1e-5
NT = 49            # dst tiles per core
NPAD = N_CORES * NT * 128
SHARD = NT * 128
CHUNK_SIZES = [1, 2, 4, 5, 6, 6, 6, 6, 5, 4, 3, 1]

# dtype of the host-expanded per-slot source rows, per layer
DUP_DT = [FP8, FP8, FP8]
DUP_NP = [NPFP8, NPFP8, NPFP8]
# dtype of the h' outputs of layers 0/1 (input precision of the next layer)
OUT_DT = [FP8, FP8]
OUT_NP = [NPFP8, NPFP8]


# ------------------------------------------------------------------ planning
def _plan(src, dst, n_true):
    """Static (h-independent) structure: renumbering, slot stream, panels."""
    degp = np.bincount(dst, minlength=NPAD).astype(np.int64) + 1
    degp[n_true:] = 0

    order = np.argsort(degp, kind="stable")  # new -> orig
    newpos = np.empty(NPAD, np.int64)
    newpos[order] = np.arange(NPAD)          # orig -> new

    # tile k budget: global tile group of 8 (one per core) shares k
    kt = np.zeros(NT, np.int64)
    for t in range(NT):
        kt[t] = degp[order[t * 1024:(t + 1) * 1024]].max()
    kt = np.maximum(kt, 1)

    # block structure per tile: block b covers dst cols [lo, lo+w)
    blocks = []   # per tile: list of (lo, w)
    pan_cols = [] # per tile: list of panel col offsets (into global panel)
    wtot = 0
    for t in range(NT):
        k = int(kt[t])
        bl = []
        for b in range(k):
            lo = (128 * b) // k
            hi = (128 * (b + 1) - 1) // k
            bl.append((lo, hi - lo + 1))
        blocks.append(bl)
        offs = []
        for lo, w in bl:
            offs.append(wtot)
            wtot += w
        pan_cols.append(offs)

    nblk = int(kt.sum())
    tile_base = np.zeros(NT + 1, np.int64)
    tile_base[1:] = np.cumsum(128 * kt)
    meta = {"kt": kt, "blocks": blocks, "pan_cols": pan_cols,
            "wtot": wtot, "nblk": nblk, "order": order, "newpos": newpos,
            "n_true": n_true, "tile_base": tile_base,
            "total_slots": int(tile_base[-1])}
    return meta


def _build_static(meta, src, dst, batch):
    """Per-core constant tables: slot->src map, per-layer panels, rows."""
    kt, blocks, pan_cols = meta["kt"], meta["blocks"], meta["pan_cols"]
    wtot, nblk, order, newpos = (meta["wtot"], meta["nblk"], meta["order"],
                                 meta["newpos"])
    n_true = meta["n_true"]

    deg = np.bincount(dst, minlength=NPAD).astype(np.float64) + 1.0
    dinv = (1.0 / np.sqrt(deg)).astype(np.float64)
    dinv_pad = dinv.copy()
    dinv_pad[n_true:] = 1.0

    # new-indexed per-node values
    dinv_new = dinv_pad[order]
    batch_pad = np.full(NPAD, 0, np.int64)
    batch_pad[:n_true] = batch
    batch_new = batch_pad[order]
    valid_new = (order < n_true)

    # d2[d] = sum over edges of dinv[s]*dinv[d] + dinv[d]^2 (full coef sum)
    sneig = np.bincount(dst, weights=dinv[src], minlength=NPAD)
    d2 = dinv_pad * (sneig + dinv_pad)       # orig indexed
    d2_new = d2[order]

    cnt = np.bincount(batch_pad[:n_true], minlength=64).astype(np.float64)
    invc = (1.0 / np.maximum(cnt, 1.0)).astype(np.float64)

    # per-core slot assignment
    s_new = newpos[src]
    d_new = newpos[dst]
    g_tile = d_new // 128                    # global tile of dst
    core_of = g_tile % N_CORES
    tloc = g_tile // N_CORES
    dloc = d_new % 128

    tile_base = meta["tile_base"]
    total_slots = meta["total_slots"]

    edge_w0 = dinv[src] * dinv_pad[dst] * dinv_pad[dst]   # L0 edge weight*sig

    cores = []
    for c in range(N_CORES):
        sel = core_of == c
        es, et, ed = s_new[sel], tloc[sel], dloc[sel]
        ew0 = edge_w0[sel]
        # order edges of each dst node consecutively
        key = et * (128 * 64) + ed
        o = np.argsort(key, kind="stable")
        es, et, ed, ew0 = es[o], et[o], ed[o], ew0[o]
        # slot position: base + dloc*k + rank within node (self slot first)
        k_of = kt[et]
        node_key = et * 128 + ed
        # rank of edge within its node
        uniq, first_idx, counts = np.unique(node_key, return_index=True,
                                            return_counts=True)
        rank = np.arange(len(node_key)) - np.repeat(first_idx, counts)
        slot = tile_base[et] + ed * k_of + 1 + rank   # +1: self slot at 0

        # own nodes of this core (new index), per (t, d)
        tt = np.arange(NT).repeat(128)
        dd = np.tile(np.arange(128), NT)
        own_new = (tt * N_CORES + np.full(NT * 128, c)) * 128 + dd
        own_valid = valid_new[own_new]
        self_slot = tile_base[tt] + dd * kt[tt]

        slotsrc = np.full(total_slots, NPAD, np.int64)  # NPAD -> zero row
        slotsrc[slot] = es
        slotsrc[self_slot[own_valid]] = own_new[own_valid]

        dv_own = dinv_new[own_new]           # dinv of (c,t,d) node
        # panel weights per slot, per layer
        w_l0 = np.zeros(total_slots, np.float64)
        w_l0[slot] = ew0                                  # dinv[s]*dinv[d]^2
        w_l0[self_slot[own_valid]] = (dv_own ** 3)[own_valid]
        col_dinv = np.repeat(dv_own, np.repeat(kt, 128))  # dinv[d] per slot
        filled = np.zeros(total_slots, bool)
        filled[slot] = True
        filled[self_slot[own_valid]] = True
        w_l1 = np.where(filled, col_dinv ** 2, 0.0)
        w_l2 = np.where(filled, col_dinv, 0.0)

        # panels [128, wtot]
        pans = []
        for wv in (w_l0, w_l1, w_l2):
            pan = np.zeros((128, wtot), np.float64)
            for t in range(NT):
                k = int(kt[t])
                for b, (lo, w) in enumerate(blocks[t]):
                    co = pan_cols[t][b]
                    sl0 = tile_base[t] + b * 128
                    ss = np.arange(sl0, sl0 + 128)
                    cc = (ss - tile_base[t]) // k - lo    # col within panel
                    ok = (cc >= 0) & (cc < w)
                    pan[np.arange(128)[ok], co + cc[ok]] = wv[ss][ok]
            pans.append(pan.astype(NPBF16))

        # packed bf16 rows: sigma_out per col, d2*sigma per col
        sig_row = np.zeros(SHARD, np.float64)
        sh_row = np.zeros(SHARD, np.float64)
        for t in range(NT):
            cols = slice(t * 128, (t + 1) * 128)
            nn = (t * N_CORES + c) * 128 + np.arange(128)
            sig_row[cols] = dinv_new[nn]
            sh_row[cols] = d2_new[nn] * dinv_new[nn]

        # pool panel [128, NT*64]
        gpan = np.zeros((128, NT * 64), np.float64)
        for t in range(NT):
            nn = (t * N_CORES + c) * 128 + np.arange(128)
            gb = batch_new[nn]
            ok = valid_new[nn]
            gpan[np.arange(128)[ok], t * 64 + gb[ok]] = invc[gb[ok]]

        cores.append({
            "slotsrc": slotsrc,
            "pans": pans,
            "sig_row": sig_row,
            "sh_row": sh_row,
            "gpan": gpan.astype(NPBF16),
        })
    return cores


def _dup_layout(h_new, slotsrc, np_dt):
    """[NPAD(+1), F] new-indexed rows -> [128, NBLK*F] slot-stream layout."""
    rows = h_new[slotsrc]                    # [total_slots, F]
    nblk = rows.shape[0] // 128
    F = rows.shape[1]
    return np.ascontiguousarray(
        rows.reshape(nblk, 128, F).transpose(1, 0, 2)
    ).reshape(128, nblk * F).astype(np_dt)


# ------------------------------------------------------------------ programs
def _build_stats_program(meta):
    """Per-core BN partial sums: [128, 2] = (sum x, sum x^2) per feature."""
    F = 128
    nc = bacc.Bacc("TRN2", target_bir_lowering=False, debug=False,
                   num_devices=N_CORES)
    xs_d = nc.dram_tensor("x_sh", [128, NT * F], FP8,
                          kind="ExternalInput").ap()
    ident_d = nc.dram_tensor("ident", [128, 128], F32,
                             kind="ExternalInput").ap()
    out_d = nc.dram_tensor("stat_part", [128, 2], F32,
                           kind="ExternalOutput").ap()
    with TileContext(nc) as tc:
        with tc.tile_pool(name="w", bufs=1) as wp, \
             tc.tile_pool(name="ps", bufs=1, space="PSUM") as pp:
            xs = wp.tile([128, NT * F], FP8, tag="xs")
            QF = 13 * F
            nc.sync.dma_start(out=xs[:, :QF], in_=xs_d[:, :QF])
            ident_s = wp.tile([128, 128], F32, tag="id")
            nc.sync.dma_start(out=ident_s[:], in_=ident_d[:])
            for q in range(1, 4):
                hi = min((13 + q * 12) * F, NT * F)
                nc.sync.dma_start(out=xs[:, hi - 12 * F:hi],
                                  in_=xs_d[:, hi - 12 * F:hi])
            ones_s = wp.tile([128, 1], FP8, tag="ones")
            nc.vector.memset(ones_s[:], 1.0)
            xtx_ps = pp.tile([128, 128], F32, tag="xtx")
            sx_ps = pp.tile([128, 1], F32, tag="sx")
            for t in range(NT):
                sl = xs[:, t * F:(t + 1) * F]
                nc.tensor.matmul(xtx_ps[:], sl, sl, start=(t == 0),
                                 stop=(t == NT - 1))
                nc.tensor.matmul(sx_ps[:], sl, ones_s[:], start=(t == 0),
                                 stop=(t == NT - 1))
            dg = wp.tile([128, 128], F32, tag="dg")
            nc.vector.tensor_tensor(dg[:], xtx_ps[:], ident_s[:],
                                    mybir.AluOpType.mult)
            o = wp.tile([128, 2], F32, tag="o")
            nc.vector.tensor_reduce(o[:, 1:2], dg[:], mybir.AxisListType.X,
                                    mybir.AluOpType.add)
            nc.vector.tensor_copy(o[:, 0:1], sx_ps[:])
            nc.sync.dma_start(out=out_d[:], in_=o[:])
    nc.compile()
    return nc


def _build_layer_program(meta, lay):
    kt, blocks, pan_cols, wtot, nblk, tile_base = (
        meta["kt"], meta["blocks"], meta["pan_cols"], meta["wtot"],
        meta["nblk"], meta["tile_base"])
    F = 128 if lay < 2 else 64     # dup row width (L2 rows pre-transformed)
    H = 128
    H2 = 64
    G = 64
    Ho = H if lay < 2 else H2
    N_true = meta["n_true"]
    dt_in = DUP_DT[lay]
    dt_out = OUT_DT[lay] if lay < 2 else None

    nc = bacc.Bacc("TRN2", target_bir_lowering=False, debug=False,
                   num_devices=N_CORES)

    def din(name, shape, dt):
        return nc.dram_tensor(name, list(shape), dt, kind="ExternalInput").ap()

    dup_d = din("dup", [128, nblk * F], dt_in)
    if lay == 0:
        PW_EXTRA = 0               # W1 travels in f32pack
    elif lay == 1:
        PW_EXTRA = H + H2          # W2 | W3
    else:
        PW_EXTRA = NT * G + 128    # gpan | identity
    pan_d = din("pan", [128, wtot + PW_EXTRA], BF16)
    # packed bf16 row constants
    if lay == 0:
        RP = 2 * SHARD + H        # sig | sh | b1
    elif lay == 1:
        RP = SHARD + H            # sig | b2
    else:
        RP = H2                   # b3
    rp_d = din("rowpack", [1, RP], BF16)
    if lay == 0:
        # sxp | exp | gamma | beta | W1(fp32)
        fp_d = din("f32pack", [128, 18 + H], F32)
    if lay == 2:
        pool_out = nc.dram_tensor("pool_part", [H2, G], F32,
                                  kind="ExternalOutput").ap()
    else:
        OW = 128 if lay == 0 else 64   # L1 outputs t2 = h2' @ W3
        h_out = nc.dram_tensor("h_out", [OW, NT * 128], dt_out,
                               kind="ExternalOutput").ap()

    chunk_tiles = []
    t0 = 0
    for cs in CHUNK_SIZES:
        chunk_tiles.append(list(range(t0, min(t0 + cs, NT))))
        t0 += cs

    with TileContext(nc) as tc:
        with contextlib.ExitStack() as ctx:
            cpool = ctx.enter_context(tc.tile_pool(name="const", bufs=1))
            dpool = ctx.enter_context(tc.tile_pool(name="dup", bufs=5))
            ppool = ctx.enter_context(tc.tile_pool(name="pan", bufs=2))

            # first chunk's data first so its transfer leads the queue
            def chunk_loads(tiles):
                ct0, ct1 = tiles[0], tiles[-1] + 1
                b0 = int(tile_base[ct0] // 128)
                b1 = int(tile_base[ct1] // 128)
                dup_sb = dpool.tile([128, (b1 - b0) * F], dt_in, tag="dup")
                nc.sync.dma_start(out=dup_sb[:], in_=dup_d[:, b0 * F:b1 * F])
                return dup_sb, b0

            pend = [chunk_loads(chunk_tiles[0])]
            pan_sb = ppool.tile([128, wtot + PW_EXTRA], BF16, tag="pan")
            PSPLIT = PW_EXTRA + pan_cols[12][0]
            nc.sync.dma_start(out=pan_sb[:, :PSPLIT], in_=pan_d[:, :PSPLIT])

            rp_s = cpool.tile([1, RP], BF16, tag="c_rp")
            nc.sync.dma_start(out=rp_s[:], in_=rp_d[:])
            if lay == 0:
                fp_s = cpool.tile([128, 18 + H], F32, tag="c_fp")
                nc.sync.dma_start(out=fp_s[:], in_=fp_d[:])
            nc.sync.dma_start(out=pan_sb[:, PSPLIT:], in_=pan_d[:, PSPLIT:])
            if lay == 0:
                sig_s = rp_s[0:1, 0:SHARD]
                sh_s = rp_s[0:1, SHARD:2 * SHARD]
                b_s = rp_s[0:1, 2 * SHARD:2 * SHARD + H]
            elif lay == 1:
                sig_s = rp_s[0:1, 0:SHARD]
                b_s = rp_s[0:1, SHARD:SHARD + H]
            else:
                b_s = rp_s[0:1, 0:H2]
            zr_s = cpool.tile([1, 256], BF16, tag="c_zr")
            nc.vector.memset(zr_s[:], 0.0)
            if lay == 0:
                w1f_s = fp_s[:, 18:18 + H]
                w_s = cpool.tile([128, H], BF16, tag="c_wt")
                rw_s = cpool.tile([1, H], BF16, tag="c_rw")
            elif lay == 1:
                w_s = pan_sb[:, 0:H]
                w3_s = pan_sb[:, H:H + H2]
            else:
                gpan_s = pan_sb[:, 0:NT * G]
                id_s = pan_sb[:, NT * G:NT * G + 128]
                ones_s = cpool.tile([1, 256], BF16, tag="c_ones")
                nc.vector.memset(ones_s[:], 1.0)

            # ---- BN statistics (layer 0) -> W~1 and shift row rw
            if lay == 0:
                with tc.tile_pool(name="ps_st", bufs=1, space="PSUM") as pst, \
                     tc.tile_pool(name="st_w", bufs=2) as stw:
                    sxp_s = fp_s[:, 0:8]
                    exp_s = fp_s[:, 8:16]
                    gam_s = fp_s[:, 16:17]
                    bet_s = fp_s[:, 17:18]
                    ex2 = stw.tile([128, 1], F32, tag="v1")
                    nc.vector.tensor_reduce(ex2[:], exp_s,
                                            mybir.AxisListType.X,
                                            mybir.AluOpType.add)
                    sx = stw.tile([128, 1], F32, tag="v0")
                    nc.vector.tensor_reduce(sx[:], sxp_s,
                                            mybir.AxisListType.X,
                                            mybir.AluOpType.add)
                    mu = stw.tile([128, 1], F32, tag="v2")
                    nc.vector.tensor_scalar_mul(mu[:], sx[:], 1.0 / N_true)
                    var = stw.tile([128, 1], F32, tag="v3")
                    nc.vector.tensor_scalar_mul(var[:], ex2[:], 1.0 / N_true)
                    mu2 = stw.tile([128, 1], F32, tag="v4")
                    nc.vector.tensor_tensor(mu2[:], mu[:], mu[:],
                                            mybir.AluOpType.mult)
                    nc.vector.tensor_tensor(var[:], var[:], mu2[:],
                                            mybir.AluOpType.subtract)
                    nc.vector.tensor_scalar_add(var[:], var[:], BN_EPS)
                    rec = stw.tile([128, 1], F32, tag="v5")
                    nc.vector.reciprocal(rec[:], var[:])
                    isd = stw.tile([128, 1], F32, tag="v6")
                    nc.scalar.activation(isd[:], rec[:],
                                         mybir.ActivationFunctionType.Sqrt)
                    a_c = stw.tile([128, 1], F32, tag="v7")
                    nc.vector.tensor_tensor(a_c[:], gam_s, isd[:],
                                            mybir.AluOpType.mult)
                    nc.vector.tensor_scalar_mul(w_s[:], w1f_s, a_c[:])
                    ca = stw.tile([128, 1], F32, tag="v8")
                    nc.vector.tensor_tensor(ca[:], mu[:], a_c[:],
                                            mybir.AluOpType.mult)
                    nc.vector.tensor_tensor(ca[:], bet_s, ca[:],
                                            mybir.AluOpType.subtract)
                    rw_ps = pst.tile([1, H], F32, tag="rw")
                    nc.tensor.matmul(rw_ps[:], ca[:], w1f_s,
                                     start=True, stop=True)
                    nc.scalar.activation(rw_s[:], rw_ps[:],
                                         mybir.ActivationFunctionType.Copy)

            spool = ctx.enter_context(tc.tile_pool(name="stg", bufs=1))
            wpool = ctx.enter_context(tc.tile_pool(name="wk", bufs=4))
            ps_agg = ctx.enter_context(
                tc.tile_pool(name="ps_agg", bufs=3, space="PSUM"))
            if lay < 2:
                ps_out = ctx.enter_context(
                    tc.tile_pool(name="ps_out", bufs=3, space="PSUM"))
            if lay == 1:
                ps_t = ctx.enter_context(
                    tc.tile_pool(name="ps_t", bufs=2, space="PSUM"))
                tstage = spool.tile([64, NT * 128], dt_out, tag="tstg")
            if lay == 2:
                ps_tr = ctx.enter_context(
                    tc.tile_pool(name="ps_tr", bufs=2, space="PSUM"))
                ps_pl = ctx.enter_context(
                    tc.tile_pool(name="ps_pl", bufs=1, space="PSUM"))
                pool_ps = ps_pl.tile([H2, G], F32, tag="pool")

            if lay == 0:
                stage = spool.tile([128, NT * 128], dt_out, tag="stg")
            elif lay == 1:
                stage = spool.tile([128, NT * 128], BF16, tag="stg")

            state = {"use_dve": False}

            def flip():
                state["use_dve"] = not state["use_dve"]
                return state["use_dve"]

            def phase1(pr, dup_sb, b0):
                """agg matmuls (+ L2: bias + relu straight from PSUM)."""
                pw = len(pr) * 128
                rows = H2 if lay == 2 else 128
                agg_ps = ps_agg.tile([rows, pw], F32, tag="agg")
                nc.tensor.matmul(agg_ps[:], zr_s[0:1, 0:rows],
                                 zr_s[0:1, 0:pw], start=True, stop=False,
                                 skip_group_check=True)
                nb_pair = sum(int(kt[t]) for t in pr)
                bi = 0
                for hi, t in enumerate(pr):
                    for b, (lo, w) in enumerate(blocks[t]):
                        gb = int(tile_base[t] // 128) + b
                        co = pan_cols[t][b]
                        bi += 1
                        nc.tensor.matmul(
                            agg_ps[:, hi * 128 + lo:hi * 128 + lo + w],
                            dup_sb[:, (gb - b0) * F:(gb - b0 + 1) * F],
                            pan_sb[:, PW_EXTRA + co:PW_EXTRA + co + w],
                            start=False,
                            stop=(bi == nb_pair and lay != 2),
                            skip_group_check=True)
                if lay == 2:
                    nc.tensor.matmul(agg_ps[:], b_s, ones_s[0:1, 0:pw],
                                     start=False, stop=True,
                                     skip_group_check=True)
                    hsT = wpool.tile([H2, pw], BF16, tag="hsT")
                    if flip():
                        nc.vector.tensor_scalar_max(hsT[:], agg_ps[:], 0.0)
                    else:
                        nc.scalar.activation(
                            hsT[:], agg_ps[:],
                            mybir.ActivationFunctionType.Relu)
                    return pr, agg_ps, hsT
                aggT = wpool.tile([128, pw], BF16, tag="aggT")
                if flip():
                    nc.vector.tensor_copy(aggT[:], agg_ps[:])
                else:
                    nc.scalar.activation(aggT[:], agg_ps[:],
                                         mybir.ActivationFunctionType.Copy)
                return pr, agg_ps, aggT

            def phase2(st1):
                pr, agg_ps, aggT = st1
                pw = len(pr) * 128
                if lay < 2:
                    h_ps = ps_out.tile([Ho, pw], F32, tag="hps")
                    for hi, t in enumerate(pr):
                        hsl = slice(hi * 128, (hi + 1) * 128)
                        nc.tensor.matmul(h_ps[:, hsl], w_s[:] if lay == 0
                                         else w_s, aggT[:, hsl],
                                         start=True, stop=False,
                                         skip_group_check=True)
                        nc.tensor.matmul(
                            h_ps[:, hsl], b_s,
                            sig_s[0:1, t * 128:(t + 1) * 128],
                            start=False, stop=(lay != 0),
                            skip_group_check=True)
                        if lay == 0:
                            nc.tensor.matmul(
                                h_ps[:, hsl], rw_s[:],
                                sh_s[0:1, t * 128:(t + 1) * 128],
                                start=False, stop=True,
                                skip_group_check=True)
                    so = pr[0] * 128
                    if flip():
                        nc.vector.tensor_scalar_max(
                            stage[:, so:so + pw], h_ps[:], 0.0)
                    else:
                        nc.scalar.activation(
                            stage[:, so:so + pw], h_ps[:],
                            mybir.ActivationFunctionType.Relu)
                    return st1
                # lay 2: transpose each tile's hsT half: [64, 128] -> [128, 64]
                hsT = aggT
                trs = []
                for hi, t in enumerate(pr):
                    tr_ps = ps_tr.tile([128, H2], BF16, tag="tr")
                    nc.tensor.transpose(tr_ps[:],
                                        hsT[:, hi * 128:(hi + 1) * 128],
                                        id_s[0:64, 0:64])
                    hs_sb = wpool.tile([128, H2], BF16, tag="hs")
                    if flip():
                        nc.vector.tensor_copy(hs_sb[:], tr_ps[:])
                    else:
                        nc.scalar.activation(
                            hs_sb[:], tr_ps[:],
                            mybir.ActivationFunctionType.Copy)
                    trs.append((t, hs_sb))
                return trs

            def phase3(st2):
                if lay == 1:
                    pr = st2[0]
                    pw = len(pr) * 128
                    so = pr[0] * 128
                    t_ps = ps_t.tile([H2, pw], F32, tag="tps")
                    nc.tensor.matmul(t_ps[:], w3_s, stage[:, so:so + pw],
                                     start=True, stop=True,
                                     skip_group_check=True)
                    if flip():
                        nc.vector.tensor_copy(tstage[:, so:so + pw], t_ps[:])
                    else:
                        nc.scalar.activation(
                            tstage[:, so:so + pw], t_ps[:],
                            mybir.ActivationFunctionType.Copy)
                elif lay == 2:
                    for t, hs_sb in st2:
                        nc.tensor.matmul(pool_ps[:], hs_sb[:],
                                         gpan_s[:, t * G:(t + 1) * G],
                                         start=(t == 0), stop=(t == NT - 1),
                                         skip_group_check=True)

            # pair pipeline across all chunks: phase1(i) | phase2(i-1) |
            # phase3(i-2) keeps PE from stalling on Act/DVE results
            all_pairs = []
            for ci, tiles in enumerate(chunk_tiles):
                dup_sb, b0 = pend.pop(0)
                if ci + 1 < len(chunk_tiles):
                    pend.append(chunk_loads(chunk_tiles[ci + 1]))
                for i in range(0, len(tiles), 2):
                    all_pairs.append((tiles[i:i + 2], dup_sb, b0))

            hastail = lay > 0
            q2, q3 = [], []
            out_stage = stage if lay == 0 else (tstage if lay == 1 else None)
            OW = 128 if lay == 0 else 64
            WRITES = [(5, 0, 24), (8, 24, 41), (11, 41, NT)]

            def maybe_write(done_tile):
                if lay == 2:
                    return
                while WRITES and done_tile >= WRITES[0][2] - 1:
                    _, wt0, wt1 = WRITES.pop(0)
                    nc.scalar.dma_start(
                        out=h_out[:, wt0 * 128:wt1 * 128],
                        in_=out_stage[:, wt0 * 128:wt1 * 128])

            def run3():
                st3 = q3.pop(0)
                phase3(st3)
                done = st3[0][-1] if lay == 1 else st3[-1][0]
                maybe_write(done)

            def run2():
                st2 = phase2(q2.pop(0))
                if hastail:
                    q3.append(st2)
                else:
                    maybe_write(st2[0][-1])

            for item in all_pairs:
                st1 = phase1(*item)
                if len(q3) >= 2:
                    run3()
                if q2:
                    run2()
                q2.append(st1)
            while q2 or q3:
                if q3:
                    run3()
                if q2:
                    run2()
            if lay == 2:
                po = wpool.tile([H2, G], F32, tag="po")
                nc.vector.tensor_copy(po[:], pool_ps[:])
                nc.sync.dma_start(out=pool_out[:], in_=po[:])

    nc.compile()
    return nc


def _build_mlp_program(meta):
    G, H2, H4, C = 64, 64, 32, 2
    nc = bacc.Bacc("TRN2", target_bir_lowering=False, debug=False,
                   num_devices=N_CORES)
    # pack1 [128, 4G + H2]: cols 0:4G = pool partials (2i in rows 0:64,
    # 2i+1 in rows 64:128), cols 4G: = stacked identity
    pk1_d = nc.dram_tensor("pack1", [128, 4 * G + H2], F32,
                           kind="ExternalInput").ap()
    # pack2 [64, 37]: wc1 | bc2b | wc2 | bc1
    pk2_d = nc.dram_tensor("pack2", [64, 37], F32,
                           kind="ExternalInput").ap()
    out_d = nc.dram_tensor("out", [G, C], F32, kind="ExternalOutput").ap()

    with TileContext(nc) as tc:
        with tc.tile_pool(name="w", bufs=1) as wp, \
             tc.tile_pool(name="ps", bufs=1, space="PSUM") as pp:
            pk1_s = wp.tile([128, 4 * G + H2], F32, tag="pk1")
            nc.sync.dma_start(out=pk1_s[:], in_=pk1_d[:])
            pk2_s = wp.tile([64, 37], F32, tag="pk2")
            nc.sync.dma_start(out=pk2_s[:], in_=pk2_d[:])
            eye2_s = pk1_s[:, 4 * G:4 * G + H2]
            wc1_s = pk2_s[:, 0:32]
            bc2_s = pk2_s[:, 32:34]
            wc2_s = pk2_s[0:32, 34:36]
            bc1_s = pk2_s[0:32, 36:37]

            acc_ps = pp.tile([H2, G], F32, tag="acc")
            for i in range(4):
                nc.tensor.matmul(acc_ps[:], eye2_s,
                                 pk1_s[:, i * G:(i + 1) * G],
                                 start=(i == 0), stop=(i == 3))
            acc_s = wp.tile([H2, G], F32, tag="accs")
            nc.vector.tensor_copy(acc_s[:], acc_ps[:])
            z_ps = pp.tile([H4, G], F32, tag="z")
            nc.tensor.matmul(z_ps[:], wc1_s, acc_s[:], start=True,
                             stop=True)
            z_s = wp.tile([H4, G], F32, tag="zs")
            nc.vector.tensor_scalar(z_s[:], z_ps[:], bc1_s, 0.0,
                                    mybir.AluOpType.add,
                                    mybir.AluOpType.max)
            o_ps = pp.tile([G, C], F32, tag="o")
            nc.tensor.matmul(o_ps[:], z_s[:], wc2_s, start=True, stop=True)
            o_s = wp.tile([G, C], F32, tag="os")
            nc.vector.tensor_tensor(o_s[:], o_ps[:], bc2_s,
                                    mybir.AluOpType.add)
            nc.sync.dma_start(out=out_d[:], in_=o_s[:])
    nc.compile()
    return nc


# ------------------------------------------------------------------ driver
_CACHE = {}


def _get_programs(meta):
    key = (tuple(meta["kt"]), meta["n_true"])
    if key not in _CACHE:
        progs = [_build_stats_program(meta)]
        progs += [_build_layer_program(meta, lay) for lay in range(3)]
        progs.append(_build_mlp_program(meta))
        _CACHE[key] = progs
    return _CACHE[key]


def run_gnn(runner=None, **inputs):
    F, H, H2, H4, C, G = 128, 128, 64, 32, 2, 64
    x = np.asarray(inputs["x"], np.float32)
    n_true = x.shape[0]
    src = np.asarray(inputs["edge_index"][0], np.int64)
    dst = np.asarray(inputs["edge_index"][1], np.int64)
    batch = np.asarray(inputs["batch"], np.int64)

    meta = _plan(src, dst, n_true)
    cores = _build_static(meta, src, dst, batch)
    order = meta["order"]
    progs = _get_programs(meta)

    def run(nc, in_maps):
        if runner is not None:
            return runner(nc, in_maps)
        return run_bass_kernel_spmd(
            nc, in_maps, core_ids=list(range(N_CORES))).results

    # x rows in new order, padded, with an extra zero row at index NPAD
    x_new = np.zeros((NPAD + 1, F), np.float32)
    x_new[:NPAD][order < n_true] = x[order[order < n_true]]

    # ---- stats launch (reads new-order x shards, tile-major per core)
    xb = x_new[:NPAD].astype(NPFP8)
    stats_maps = []
    for c in range(N_CORES):
        idx = ((np.arange(NT) * N_CORES + c)[:, None] * 128
               + np.arange(128)[None, :])          # [NT, 128] node ids
        slab = xb[idx]                             # [NT, 128, F]
        slab = np.ascontiguousarray(slab.transpose(1, 0, 2)).reshape(
            128, NT * F)
        stats_maps.append({"x_sh": slab,
                           "ident": np.eye(128, dtype=np.float32)})
    res = run(progs[0], stats_maps)
    parts = np.stack([np.asarray(res[c]["stat_part"])
                      for c in range(N_CORES)], axis=2)
    sx_parts = np.ascontiguousarray(parts[:, 0, :], dtype=np.float32)
    ex2_parts = np.ascontiguousarray(parts[:, 1, :], dtype=np.float32)

    W = [np.asarray(inputs["W1"], np.float32),
         np.asarray(inputs["W2"], np.float32),
         np.asarray(inputs["W3"], np.float32)]
    brows = [np.asarray(inputs["b1"], np.float32).reshape(1, H),
             np.asarray(inputs["b2"], np.float32).reshape(1, H),
             np.asarray(inputs["b3"], np.float32).reshape(1, H2)]

    h_new = x_new
    pool_parts = None
    for lay in range(3):
        maps = []
        for c in range(N_CORES):
            st = cores[c]
            if lay == 0:
                rp = np.concatenate([st["sig_row"], st["sh_row"],
                                     brows[0].ravel()])
            elif lay == 1:
                rp = np.concatenate([st["sig_row"], brows[1].ravel()])
            else:
                rp = brows[2].ravel()
            pre = []
            if lay == 1:
                pre.append(W[1].astype(NPBF16))
                pre.append(W[2].astype(NPBF16))
            if lay == 2:
                pre.append(st["gpan"])
                pre.append(np.eye(128, dtype=NPBF16))
            pan = np.concatenate(pre + [st["pans"][lay]], axis=1) \
                if pre else st["pans"][lay]
            m = {"dup": _dup_layout(h_new, st["slotsrc"], DUP_NP[lay]),
                 "pan": np.ascontiguousarray(pan),
                 "rowpack": rp.astype(NPBF16).reshape(1, -1)}
            if lay == 0:
                fp = np.zeros((128, 18 + H), np.float32)
                fp[:, 0:8] = sx_parts
                fp[:, 8:16] = ex2_parts
                fp[:, 16] = np.asarray(inputs["bn_gamma"], np.float32)
                fp[:, 17] = np.asarray(inputs["bn_beta"], np.float32)
                fp[:, 18:] = W[0]
                m["f32pack"] = fp
            maps.append(m)
        res = run(progs[1 + lay], maps)
        if lay < 2:
            # h_out [OW, NT*128] per core -> h_new [NPAD+1, OW]
            OW = 128 if lay == 0 else 64
            h_new = np.zeros((NPAD + 1, OW), np.float32)
            for c in range(N_CORES):
                ho = np.asarray(res[c]["h_out"])   # [OW, NT*128]
                hoT = ho.reshape(OW, NT, 128).transpose(1, 2, 0)
                idx = ((np.arange(NT) * N_CORES + c)[:, None] * 128
                       + np.arange(128)[None, :])
                h_new[idx] = hoT
        else:
            pool_parts = [np.asarray(res[c]["pool_part"])
                          for c in range(N_CORES)]

    # ---- MLP launch
    pk1 = np.zeros((128, 4 * G + H2), np.float32)
    for i in range(4):
        pk1[0:H2, i * G:(i + 1) * G] = pool_parts[2 * i]
        pk1[H2:128, i * G:(i + 1) * G] = pool_parts[2 * i + 1]
    pk1[0:H2, 4 * G:] = np.eye(H2, dtype=np.float32)
    pk1[H2:128, 4 * G:] = np.eye(H2, dtype=np.float32)
    pk2 = np.zeros((64, 37), np.float32)
    pk2[:, 0:32] = np.asarray(inputs["Wc1"], np.float32)
    pk2[:, 32:34] = np.tile(np.asarray(inputs["bc2"], np.float32)[None, :],
                            (G, 1))
    pk2[0:32, 34:36] = np.asarray(inputs["Wc2"], np.float32)
    pk2[0:32, 36] = np.asarray(inputs["bc1"], np.float32)
    mlp_map = {"pack1": pk1, "pack2": pk2}
    res = run(progs[4], [dict(mlp_map) for _ in range(N_CORES)])
    return np.asarray(res[0]["out"], np.float32)


def kernel(**inputs):
    return run_gnn(**inputs)
